# revision 20
# baseline (speedup 1.0000x reference)
# Causal self-attention kernel for 8 Trainium2 NeuronCores (Bass/Tile).
#
# Problem: x:(2,4096,768) f32, 12 heads, head_dim 64, causal mask, torch-Linear
# Q/K/V/out projections. out = softmax(QK^T/8, causal) V @ Wp^T + biases.
#
# Sharding: core i = batch i//4, head group i%4 (3 heads). All transposes,
# weight folds and dtype casts are done host-side in numpy; the device sees
# contraction-major bf16 operands and DMAs them straight into SBUF.
#
# Device pipeline per core (emission is hand-interleaved so QKV projection
# chunks fill PE slack between attention query-groups, and one shared PSUM
# pool keeps buffer reuse in timeline order):
#   QKV: bf16 projections; Q^T/K^T are cast to fp8 into a 256-slot layout
#     (3 heads x 64 d; K zero-padded per head) for dual-fp8 S matmuls.
#   Attention per head / 1024-query group / 128-key chunk pair:
#     S^T = K_h Q^T as dual-fp8 DoubleRow matmuls (2x PE throughput),
#     P^T = exp(S^T/8) on ACT (bf16 out), causal trim via gpsimd triangle
#     masks, PSUM-accumulate [V|1]^T P^T, divide by the ones-row sum
#     (DVE recip + DRAM-bounce broadcast + multiply).
#   One AllToAll per head re-shards A^T to query-column split, overlapped
#   with the remaining heads' attention.
#   Proj: 8 row tiles x 7 chunks (chunk 6 = ones-row bias, so no bias add);
#   PSUM->SBUF copies alternate ACT/DVE; out DMA alternates HWDGE queues.
import numpy as np
import ml_dtypes

import concourse.bass as bass  # noqa: F401
import concourse.mybir as mybir
import concourse.tile as tile
from concourse import bacc
from concourse.bass_utils import run_bass_kernel_spmd

F32 = mybir.dt.float32
BF16 = mybir.dt.bfloat16
F8 = mybir.dt.float8e4
DR = mybir.MatmulPerfMode.DoubleRow
BF16NP = ml_dtypes.bfloat16

B, T, C, H, D = 2, 4096, 768, 12, 64
NCORES = 8
GROUPS = 4              # cores per batch
HPC = H // GROUPS       # 3 heads per core
JC = HPC * D            # 192 projected features per core
P = 128
CCHUNKS = 6             # contraction chunks of C
RCHUNKS = T // P        # 32
QCW = 512               # psum bank width (f32)
NQC = T // QCW          # 8
QGW = 1024              # attention query-group width
NQG = T // QGW          # 4
KPG = QGW // P          # key chunks per query-group span (8)
ROWS_OUT = T // GROUPS  # 1024 output rows per core
SCALE = 1.0 / 8.0

_CACHE: dict = {}
LAST_RESULTS = None


def _build():
    nc = bacc.Bacc("TRN2", target_bir_lowering=False, debug=False,
                   num_devices=NCORES)

    xbt = nc.dram_tensor("xbt", [P, CCHUNKS, T], BF16, kind="ExternalInput").ap()
    wqa = nc.dram_tensor("wqa", [P, CCHUNKS, P], BF16, kind="ExternalInput").ap()
    wka = nc.dram_tensor("wka", [P, CCHUNKS, P], BF16, kind="ExternalInput").ap()
    wqb = nc.dram_tensor("wqb", [P, CCHUNKS, D], BF16, kind="ExternalInput").ap()
    wkb = nc.dram_tensor("wkb", [P, CCHUNKS, D], BF16, kind="ExternalInput").ap()
    wva = nc.dram_tensor("wva", [P, CCHUNKS, JC], BF16, kind="ExternalInput").ap()
    wpt = nc.dram_tensor("wpt", [P, 7, C], BF16, kind="ExternalInput").ap()
    bqa = nc.dram_tensor("bqa", [P], F32, kind="ExternalInput").ap()
    bqb = nc.dram_tensor("bqb", [D], F32, kind="ExternalInput").ap()
    bka = nc.dram_tensor("bka", [P], F32, kind="ExternalInput").ap()
    bkb = nc.dram_tensor("bkb", [D], F32, kind="ExternalInput").ap()
    bv = nc.dram_tensor("bv", [JC], F32, kind="ExternalInput").ap()
    tri = nc.dram_tensor("tri", [P, P], BF16, kind="ExternalInput").ap()
    mask2 = nc.dram_tensor("mask2", [P, 2 * P], BF16, kind="ExternalInput").ap()
    out = nc.dram_tensor("out_part", [ROWS_OUT, C], F32,
                         kind="ExternalOutput").ap()

    with tile.TileContext(nc) as tc, \
         tc.tile_pool(name="persist", bufs=1) as persist, \
         tc.tile_pool(name="att_sb", bufs=4) as att_sb, \
         tc.tile_pool(name="div_sb", bufs=3) as div_sb, \
         tc.tile_pool(name="div_dram", bufs=3, space="DRAM") as div_dram, \
         tc.tile_pool(name="a2a_dram", bufs=1, space="DRAM") as a2a_dram, \
         tc.tile_pool(name="proj_sb", bufs=4) as proj_sb:

        def ptile(shape, dtype, name):
            return persist.tile(shape, dtype, name=name, tag=name)

        # ---------- persistent SBUF ----------
        xbT = ptile([P, CCHUNKS, T], BF16, name="xbT")
        wqa_s = ptile([P, CCHUNKS, P], BF16, name="wqa_s")
        wka_s = ptile([P, CCHUNKS, P], BF16, name="wka_s")
        wqb_s = ptile([P, CCHUNKS, D], BF16, name="wqb_s")
        wkb_s = ptile([P, CCHUNKS, D], BF16, name="wkb_s")
        wva_s = ptile([P, CCHUNKS, JC], BF16, name="wva_s")
        wpt_s = ptile([P, 7, C], BF16, name="wpt_s")
        trimask = ptile([P, P], BF16, name="trimask")
        m2mask = ptile([P, 2 * P], BF16, name="m2mask")
        bqa_t = ptile([P, 1], F32, name="bqa_t")
        bqb_t = ptile([D, 1], F32, name="bqb_t")
        bka_t = ptile([P, 1], F32, name="bka_t")
        bkb_t = ptile([D, 1], F32, name="bkb_t")
        bv_bc = ptile([P, JC], F32, name="bv_bc")

        # Q^T/K^T in dual-fp8 256-slot layout: slot s = 64*h + d at
        # (i = s//128, p = s%128).  qtf shared; ktf zero-padded per head.
        qtf = ptile([P, 2, T], F8, name="qtf")
        ktf = [ptile([P, 2, T], F8, name=f"ktf{h}") for h in range(HPC)]
        vones = ptile([P, RCHUNKS, HPC, D + 1], BF16, name="vones")
        # agT per head: [p = 64*half + d, b2, sp, q]; plus a bias tile
        agTh = [ptile([P, 2, 2, QCW], BF16, name=f"agT{h}")
                for h in range(HPC)]
        agTb = ptile([P, 2, QCW], BF16, name="agTb")

        # input loads: all on the scalar HWDGE queue, ordered by first use;
        # xbt column blocks are interleaved so QKV can start early
        def load_xbt(qc):
            cs = slice(qc * QCW, (qc + 1) * QCW)
            nc.scalar.dma_start(xbT[:, :, cs], xbt[:, :, cs])

        nc.scalar.dma_start(wqa_s, wqa)
        nc.scalar.dma_start(wqb_s, wqb)
        load_xbt(0)
        nc.sync.dma_start(wka_s, wka)
        nc.sync.dma_start(wkb_s, wkb)
        nc.sync.dma_start(bka_t, bka[:, None])
        nc.sync.dma_start(bkb_t, bkb[:, None])
        nc.sync.dma_start(bqa_t, bqa[:, None])
        nc.sync.dma_start(bqb_t, bqb[:, None])
        nc.sync.dma_start(bv_bc, bv[None, :].to_broadcast((P, JC)))
        nc.sync.dma_start(trimask, tri)
        nc.sync.dma_start(m2mask, mask2)
        load_xbt(1)
        nc.scalar.dma_start(wva_s, wva)
        for qc in range(2, NQC):
            load_xbt(qc)

        # zero the dead fp8 slots one column-quarter at a time; quarter q
        # is emitted just before the first attention block that reads it so
        # the Pool mask muls are not stuck behind a long memset queue
        def memset_quarter(colq):
            cs = slice(colq * (T // 4), (colq + 1) * (T // 4))
            nc.gpsimd.memset(ktf[0][D:P, 0, cs], 0.0)
            nc.gpsimd.memset(ktf[0][:, 1, cs], 0.0)
            nc.gpsimd.memset(ktf[1][0:D, 0, cs], 0.0)
            nc.gpsimd.memset(ktf[1][:, 1, cs], 0.0)
            nc.gpsimd.memset(ktf[2][:, 0, cs], 0.0)
            nc.gpsimd.memset(ktf[2][D:P, 1, cs], 0.0)
            nc.gpsimd.memset(qtf[D:P, 1, cs], 0.0)

        memset_quarter(0)
        nc.gpsimd.memset(vones[:, :, :, D:D + 1], 1.0)
        nc.gpsimd.memset(agTb, 0.0)
        nc.gpsimd.memset(agTb[0:1, :, :], 1.0)

        # pre-warm the ACT exp table
        warm = div_sb.tile([P, 1], F32, name="warm", tag="warm")
        nc.scalar.activation(warm, bqa_t, mybir.ActivationFunctionType.Exp)

        a2a_in = [a2a_dram.tile([NCORES, D, QCW], BF16, name=f"a2a_in{h}",
                                tag=f"a2a_in{h}") for h in range(HPC)]
        a2a_out = [a2a_dram.tile([NCORES * D * QCW, 1], BF16,
                                 name=f"a2a_out{h}", tag=f"a2a_out{h}")
                   for h in range(HPC)]

        # one shared PSUM pool: tags cycle in emission (=timeline) order
        ps = tc.alloc_tile_pool(name="ps", bufs=2, space="PSUM")

        # ---- QKV micro-units, injected into attention kc-slots ----
        def qa_unit(qc):
            cs = slice(qc * QCW, (qc + 1) * QCW)

            def emit():
                t = ps.tile([P, QGW], F32, name="tu", tag="pss")
                psq = t[:, 0:QCW]
                for cc in range(CCHUNKS):
                    nc.tensor.matmul(psq, wqa_s[:, cc, :], xbT[:, cc, cs],
                                     start=(cc == 0), stop=(cc == CCHUNKS - 1))
                nc.vector.tensor_scalar_add(qtf[:, 0, cs], psq, bqa_t)
            return emit

        def qb_unit(qc):
            cs = slice(qc * QCW, (qc + 1) * QCW)

            def emit():
                t = ps.tile([P, QGW], F32, name="tu", tag="pss")
                psq = t[0:D, 0:QCW]
                for cc in range(CCHUNKS):
                    nc.tensor.matmul(psq, wqb_s[:, cc, :], xbT[:, cc, cs],
                                     start=(cc == 0), stop=(cc == CCHUNKS - 1))
                nc.vector.tensor_scalar_add(qtf[0:D, 1, cs], psq, bqb_t)
            return emit

        def kb_unit(qc):
            # head-2's K tail: only needed once (2, *) blocks run
            cs = slice(qc * QCW, (qc + 1) * QCW)

            def emit():
                t = ps.tile([P, QGW], F32, name="tu", tag="pss")
                psk = t[0:D, 0:QCW]
                for cc in range(CCHUNKS):
                    nc.tensor.matmul(psk, wkb_s[:, cc, :], xbT[:, cc, cs],
                                     start=(cc == 0), stop=(cc == CCHUNKS - 1))
                nc.vector.tensor_scalar_add(ktf[2][0:D, 1, cs], psk, bkb_t)
            return emit

        def ka_unit(qc):
            cs = slice(qc * QCW, (qc + 1) * QCW)

            def emit():
                t = ps.tile([P, QGW], F32, name="tu", tag="pss")
                psk = t[:, 0:QCW]
                for cc in range(CCHUNKS):
                    nc.tensor.matmul(psk, wka_s[:, cc, :], xbT[:, cc, cs],
                                     start=(cc == 0),
                                     stop=(cc == CCHUNKS - 1))
                nc.vector.tensor_scalar_add(ktf[0][0:D, 0, cs],
                                            psk[0:D, :], bka_t[0:D])
                nc.vector.tensor_scalar_add(ktf[1][D:P, 0, cs],
                                            psk[D:P, :], bka_t[D:P])
            return emit

        def v_unit(rc):
            def emit():
                t = ps.tile([P, QGW], F32, name="tu", tag="pss")
                psv = t[:, 0:JC]
                for cc in range(CCHUNKS):
                    nc.tensor.matmul(psv, xbT[:, cc, rc * P:(rc + 1) * P],
                                     wva_s[:, cc, :], start=(cc == 0),
                                     stop=(cc == CCHUNKS - 1))
                nc.vector.tensor_add(
                    vones[:, rc, :, 0:D],
                    psv.rearrange("p (h d) -> p h d", h=HPC),
                    bv_bc.rearrange("p (h d) -> p h d", h=HPC))
            return emit

        class Feeder:
            """Deadline-sorted QKV unit queue; advance() emits overdue units
            plus at most ~one slot's worth of PE slack (budget in ns) of
            soon-due units, so PE bursts never starve ACT."""

            def __init__(self):
                self.units = []  # (deadline_slot, cost_ns, emit_fn)
                self.i = 0

            def seal(self):
                self.units.sort(key=lambda u: u[0])

            def advance(self, slot, budget=800.0, horizon=12):
                spent = 0.0
                while self.i < len(self.units):
                    dl, cost, emit = self.units[self.i]
                    if dl >= slot and (dl > slot + horizon or spent >= budget):
                        break
                    emit()
                    spent += cost
                    self.i += 1

            def flush(self):
                while self.i < len(self.units):
                    self.units[self.i][2]()
                    self.i += 1

        feeder = Feeder()

        pending_div = [None]

        def flush_div():
            if pending_div[0] is not None:
                pending_div[0]()
                pending_div[0] = None

        def emit_att(h, qg, slot0=None):
            pso = ps.tile([D + 1, QGW], F32, name="pso", tag="pso")
            nkc = (qg + 1) * KPG
            diag0 = qg * KPG

            def emit_pv(kc_e, qoff, pT):
                for half in range(2):
                    kc = kc_e + half
                    for sub in range(QGW // QCW):
                        lo, hi = max(qoff, sub * QCW), (sub + 1) * QCW
                        if lo >= hi:
                            continue
                        nc.tensor.matmul(
                            pso[:, lo:hi], vones[:, kc, h, :],
                            pT[:, half, lo:hi],
                            start=(kc == 0), stop=(kc == nkc - 1))

            pending = None  # PV is emitted one pair late so PE is never
            for kcp in range(nkc // 2):  # stuck waiting on the Pool masks
                kc_e = 2 * kcp
                if kcp == 2:
                    flush_div()  # prev block's atile mul: its DRAM-bounced
                    # broadcast has landed by now, so DVE does not stall
                if slot0 is not None:
                    feeder.advance(slot0 + kc_e)
                qoff = max(0, (kc_e - diag0) * P)
                pT = att_sb.tile([P, 2, QGW], BF16, name="pT", tag="pT")
                for half in range(2):
                    kc = kc_e + half
                    pss = ps.tile([P, QGW], F32, name="pss", tag="pss")
                    for sub in range(QGW // QCW):
                        lo, hi = max(qoff, sub * QCW), (sub + 1) * QCW
                        if lo >= hi:
                            continue
                        nc.tensor.matmul(
                            pss[:, lo:hi], ktf[h][:, :, kc * P:(kc + 1) * P],
                            qtf[:, :, qg * QGW + lo:qg * QGW + hi],
                            start=True, stop=True, perf_mode=DR)
                    nc.scalar.activation(
                        pT[:, half, qoff:QGW], pss[:, qoff:QGW],
                        mybir.ActivationFunctionType.Exp, scale=SCALE)
                if kc_e >= diag0:
                    # causal trim: even chunk triangle; odd chunk
                    # [zeros | triangle] over 256 cols
                    nc.gpsimd.tensor_mul(pT[:, 0, qoff:qoff + P],
                                         pT[:, 0, qoff:qoff + P], trimask)
                    nc.gpsimd.tensor_mul(pT[:, 1, qoff:qoff + 2 * P],
                                         pT[:, 1, qoff:qoff + 2 * P], m2mask)
                if pending is not None:
                    emit_pv(*pending)
                pending = (kc_e, qoff, pT)
            emit_pv(*pending)
            recip = div_sb.tile([1, QGW], F32, name="recip", tag="recip")
            nc.vector.reciprocal(recip, pso[D:D + 1, :])
            rdram = div_dram.tile([1, QGW], F32, name="rdram", tag="rdram")
            nc.sync.dma_start(rdram, recip)
            rbc = div_sb.tile([D, QGW], F32, name="rbc", tag="rbc")
            nc.sync.dma_start(rbc, rdram.to_broadcast((D, QGW)))

            def div_mul(h=h, qg=qg, pso=pso, rbc=rbc):
                atile = div_sb.tile([D, QGW], BF16, name="atile", tag="atile")
                nc.vector.tensor_mul(atile, pso[0:D, :], rbc)
                for half in range(2):
                    nc.sync.dma_start(
                        a2a_in[h][2 * qg + half, :, :],
                        atile[:, half * QCW:(half + 1) * QCW])
            pending_div[0] = div_mul

        def emit_a2a(h):
            nc.gpsimd.collective_compute(
                "AllToAll", mybir.AluOpType.bypass,
                replica_groups=[list(range(NCORES))],
                ins=[a2a_in[h].opt()], outs=[a2a_out[h].opt()])
            for b2 in range(2):
                for sp in range(2):
                    s0 = 4 * b2 + 2 * sp
                    nc.sync.dma_start(
                        agTh[h][:, b2, sp, :],
                        a2a_out[h][s0 * D * QCW:(s0 + 2) * D * QCW, 0]
                        .rearrange("(p q) -> p q", q=QCW))

        # ---- schedule: prefix QKV for the first query group, then inject
        # the remaining QKV micro-units into attention kc-slots just before
        # their deadline, so PE fills ACT-bound attention slack ----
        # PE p-state warm-up: ~4us of throwaway matmuls that only depend on
        # the first weight load, so the real prefix is costed at full clock
        jt = ps.tile([P, QGW], F32, name="jt", tag="pss")
        for _ in range(24):
            nc.tensor.matmul(jt[:, 0:P], wqa_s[:, 0, :], wqa_s[:, 0, :],
                             start=True, stop=True)

        for qc in (0, 1):
            qa_unit(qc)()
            qb_unit(qc)()
            ka_unit(qc)()
        v_unit(0)()
        v_unit(1)()

        # block order: interleave heads so head-0's QKV deadlines spread over
        # many slots; two head-2 blocks run before (0,3) for extra slack
        order = [(0, 0), (1, 0), (0, 1), (1, 1), (0, 2), (1, 2), (2, 0),
                 (2, 1), (0, 3), "a2a0", (1, 3), "a2a1", (2, 2), (2, 3),
                 "a2a2"]
        start_slot = {}
        s = 0
        for blk in order:
            if isinstance(blk, tuple):
                start_slot[blk] = s
                s += (blk[1] + 1) * KPG
        QK_NS, V_NS = 1278.0, 480.0
        for qc in range(2, NQC):
            # stagger the Q units backward from the block-start deadline
            dl = start_slot[(0, qc // 2)] - 1
            feeder.units.append((dl - 6 * (1 - qc % 2) - 3, QK_NS,
                                 qa_unit(qc)))
            feeder.units.append((dl - 6 * (1 - qc % 2), QK_NS,
                                 qb_unit(qc)))
            feeder.units.append((start_slot[(0, qc // 2)] + 4 * qc - 1, QK_NS,
                                 ka_unit(qc)))
        for qc in range(NQC):
            feeder.units.append((start_slot[(2, qc // 2)] + 4 * qc - 1, QK_NS,
                                 kb_unit(qc)))
        for rc in range(2, RCHUNKS):
            feeder.units.append((start_slot[(0, rc // KPG)] + rc - 1, V_NS,
                                 v_unit(rc)))
        feeder.seal()

        for blk in order:
            if blk == "a2a0":
                flush_div()
                emit_a2a(0)
            elif blk == "a2a1":
                flush_div()
                emit_a2a(1)
            elif blk == "a2a2":
                flush_div()
                emit_a2a(2)
            else:
                h, qg = blk
                if blk[0] == 0 and blk[1] >= 1:
                    memset_quarter(blk[1])
                feeder.advance(start_slot[blk] - 1)
                emit_att(h, qg, slot0=start_slot[blk])
        feeder.flush()
        ps.release()

        # wpt is only needed by the projection; load it out of the hot path
        nc.scalar.dma_start(wpt_s, wpt)

        # ---------- output projection (bias via ones-row chunk 6) ----------
        ps_pj = tc.alloc_tile_pool(name="ps_pj", bufs=2, space="PSUM")
        tiles = [(b2, rc) for b2 in range(2) for rc in range(QCW // P)]

        # pass A: bias + heads 0/1 chunks -> bf16 partials, overlapped with
        # the last division chain and the final AllToAll
        partials = []
        for t_i, (b2, rc) in enumerate(tiles):
            psj = ps_pj.tile([P, C], F32, name="psjA", tag="psj")
            qs = slice(rc * P, (rc + 1) * P)
            for ki, k in enumerate([6, 0, 1, 2, 3]):
                st, sp = (ki == 0), (ki == 4)
                lhsT = (agTb[:, b2, qs] if k == 6
                        else agTh[k // 2][:, b2, k % 2, qs])
                for lo, hi in ((0, QCW), (QCW, C)):
                    nc.tensor.matmul(psj[:, lo:hi], lhsT,
                                     wpt_s[:, k, lo:hi], start=st, stop=sp)
            part = persist.tile([P, C], BF16, name=f"part{t_i}",
                                tag=f"part{t_i}")
            if t_i % 2:
                nc.scalar.copy(part, psj)
            else:
                nc.vector.tensor_copy(part, psj)
            partials.append(part)

        # keep PE's p-state hot through the collective flight
        jt2 = ps_pj.tile([P, C], F32, name="jt2", tag="psj")
        for _ in range(125):
            nc.tensor.matmul(jt2[:, 0:QCW], wpt_s[:, 0, 0:P],
                             wpt_s[:, 0, 0:QCW], start=True, stop=True)

        # pass B: head-2 chunks + partial + store
        for t_i, (b2, rc) in enumerate(tiles):
            psj = ps_pj.tile([P, C], F32, name="psjB", tag="psj")
            qs = slice(rc * P, (rc + 1) * P)
            for ki, k in enumerate([4, 5]):
                st, sp = (ki == 0), (ki == 1)
                lhsT = agTh[2][:, b2, k % 2, qs]
                for lo, hi in ((0, QCW), (QCW, C)):
                    nc.tensor.matmul(psj[:, lo:hi], lhsT,
                                     wpt_s[:, k, lo:hi], start=st, stop=sp)
            osb = proj_sb.tile([P, C], F32, name="osb", tag="osb")
            nc.vector.tensor_add(osb, psj, partials[t_i])
            row0 = b2 * QCW + rc * P
            eng = nc.sync if t_i % 2 else nc.scalar
            eng.dma_start(out[row0:row0 + P, :], osb)
        ps_pj.release()

    nc.compile()
    return nc


def _prep_core_inputs(x, Wq, Wk, Wv, Wp, bq, bk, bv, bp):
    """Host-side transposes/folds shared across cores, then per-core dicts."""
    xbt = []
    for b in range(B):
        xt = x[b].T.reshape(CCHUNKS, P, T).transpose(1, 0, 2)
        xbt.append(np.ascontiguousarray(xt.astype(BF16NP)))

    def fold_w(w):  # w [features, C] -> [128, CCHUNKS, features]
        wt = w.T.reshape(CCHUNKS, P, w.shape[0]).transpose(1, 0, 2)
        return np.ascontiguousarray(wt.astype(BF16NP))

    tri_np = np.triu(np.ones((P, P), dtype=np.float32)).astype(BF16NP)
    mask2_np = np.concatenate(
        [np.zeros((P, P), dtype=np.float32),
         np.triu(np.ones((P, P), dtype=np.float32))], axis=1).astype(BF16NP)

    in_maps = []
    for core in range(NCORES):
        b, hg = core // GROUPS, core % GROUPS
        js = slice(JC * hg, JC * (hg + 1))
        wq_c, wk_c, wv_c = Wq[js], Wk[js], Wv[js]
        # wpt: [p = 64*half + d, k, c]; k = 2*h_local + sp ->
        #   global head g = 3*(2*sp+half) + h_local (within the batch);
        #   chunk 6 row 0 = bp.
        wpt = np.zeros((P, 7, C), dtype=np.float32)
        for k in range(6):
            h_local, sp = k // 2, k % 2
            for half in range(2):
                g = HPC * (2 * sp + half) + h_local
                wpt[half * D:(half + 1) * D, k, :] = Wp[:, D * g:D * (g + 1)].T
        wpt[0, 6, :] = bp
        in_maps.append({
            "xbt": xbt[b],
            "wqa": fold_w(wq_c[0:P]),
            "wka": fold_w(wk_c[0:P]),
            "wqb": fold_w(wq_c[P:JC]), "wkb": fold_w(wk_c[P:JC]),
            "wva": fold_w(wv_c),
            "wpt": np.ascontiguousarray(wpt.astype(BF16NP)),
            "bqa": np.ascontiguousarray(bq[js][0:P]),
            "bqb": np.ascontiguousarray(bq[js][P:JC]),
            "bka": np.ascontiguousarray(bk[js][0:P]),
            "bkb": np.ascontiguousarray(bk[js][P:JC]),
            "bv": np.ascontiguousarray(bv[js]),
            "tri": tri_np, "mask2": mask2_np,
        })
    return in_maps


def kernel(**inputs) -> np.ndarray:
    global LAST_RESULTS
    f32 = lambda k: np.ascontiguousarray(np.asarray(inputs[k], dtype=np.float32))
    x, Wq, Wk, Wv, Wp = f32("x"), f32("Wq"), f32("Wk"), f32("Wv"), f32("Wp")
    bq, bk, bv, bp = f32("bq"), f32("bk"), f32("bv"), f32("bp")

    if "nc" not in _CACHE:
        _CACHE["nc"] = _build()
    nc = _CACHE["nc"]

    in_maps = _prep_core_inputs(x, Wq, Wk, Wv, Wp, bq, bk, bv, bp)
    res = run_bass_kernel_spmd(nc, in_maps, core_ids=list(range(NCORES)))
    LAST_RESULTS = res

    out = np.empty((B, T, C), dtype=np.float32)
    for core in range(NCORES):
        part = res.results[core]["out_part"]
        out[0, core * QCW:(core + 1) * QCW, :] = part[:QCW]
        out[1, core * QCW:(core + 1) * QCW, :] = part[QCW:]
    return out


# revision 23
# speedup vs baseline: 1.0208x; 1.0208x over previous
# Causal self-attention kernel for 8 Trainium2 NeuronCores (Bass/Tile).
#
# Problem: x:(2,4096,768) f32, 12 heads, head_dim 64, causal mask, torch-Linear
# Q/K/V/out projections. out = softmax(QK^T/8, causal) V @ Wp^T + biases.
#
# Sharding: core i = batch i//4, head group i%4 (3 heads). All transposes,
# weight folds and dtype casts are done host-side in numpy; the device sees
# contraction-major bf16 operands and DMAs them straight into SBUF.
#
# Device pipeline per core (emission is hand-interleaved so QKV projection
# chunks fill PE slack between attention query-groups, and one shared PSUM
# pool keeps buffer reuse in timeline order):
#   QKV: bf16 projections; Q^T/K^T are cast to fp8 into a 256-slot layout
#     (3 heads x 64 d; K zero-padded per head) for dual-fp8 S matmuls.
#   Attention per head / 1024-query group / 128-key chunk pair:
#     S^T = K_h Q^T as dual-fp8 DoubleRow matmuls (2x PE throughput),
#     P^T = exp(S^T/8) on ACT (bf16 out), causal trim via gpsimd triangle
#     masks, PSUM-accumulate [V|1]^T P^T, divide by the ones-row sum
#     (DVE recip + DRAM-bounce broadcast + multiply).
#   One AllToAll per head re-shards A^T to query-column split, overlapped
#   with the remaining heads' attention.
#   Proj: 8 row tiles x 7 chunks (chunk 6 = ones-row bias, so no bias add);
#   PSUM->SBUF copies alternate ACT/DVE; out DMA alternates HWDGE queues.
import numpy as np
import ml_dtypes

import concourse.bass as bass  # noqa: F401
import concourse.mybir as mybir
import concourse.tile as tile
from concourse import bacc
from concourse.bass_utils import run_bass_kernel_spmd

F32 = mybir.dt.float32
BF16 = mybir.dt.bfloat16
F8 = mybir.dt.float8e4
DR = mybir.MatmulPerfMode.DoubleRow
BF16NP = ml_dtypes.bfloat16

B, T, C, H, D = 2, 4096, 768, 12, 64
NCORES = 8
GROUPS = 4              # cores per batch
HPC = H // GROUPS       # 3 heads per core
JC = HPC * D            # 192 projected features per core
P = 128
CCHUNKS = 6             # contraction chunks of C
RCHUNKS = T // P        # 32
QCW = 512               # psum bank width (f32)
NQC = T // QCW          # 8
QGW = 1024              # attention query-group width
NQG = T // QGW          # 4
KPG = QGW // P          # key chunks per query-group span (8)
ROWS_OUT = T // GROUPS  # 1024 output rows per core
SCALE = 1.0 / 8.0

_CACHE: dict = {}
LAST_RESULTS = None


def _build():
    nc = bacc.Bacc("TRN2", target_bir_lowering=False, debug=False,
                   num_devices=NCORES)

    xbt = nc.dram_tensor("xbt", [P, CCHUNKS, T], BF16, kind="ExternalInput").ap()
    wqa = nc.dram_tensor("wqa", [P, CCHUNKS, P], BF16, kind="ExternalInput").ap()
    wka = nc.dram_tensor("wka", [P, CCHUNKS, P], BF16, kind="ExternalInput").ap()
    wqb = nc.dram_tensor("wqb", [P, CCHUNKS, D], BF16, kind="ExternalInput").ap()
    wkb = nc.dram_tensor("wkb", [P, CCHUNKS, D], BF16, kind="ExternalInput").ap()
    wva = nc.dram_tensor("wva", [P, CCHUNKS, JC], BF16, kind="ExternalInput").ap()
    wpt = nc.dram_tensor("wpt", [P, 7, C], BF16, kind="ExternalInput").ap()
    bqa = nc.dram_tensor("bqa", [P], F32, kind="ExternalInput").ap()
    bqb = nc.dram_tensor("bqb", [D], F32, kind="ExternalInput").ap()
    bka = nc.dram_tensor("bka", [P], F32, kind="ExternalInput").ap()
    bkb = nc.dram_tensor("bkb", [D], F32, kind="ExternalInput").ap()
    bv = nc.dram_tensor("bv", [JC], F32, kind="ExternalInput").ap()
    tri = nc.dram_tensor("tri", [P, P], BF16, kind="ExternalInput").ap()
    mask2 = nc.dram_tensor("mask2", [P, 2 * P], BF16, kind="ExternalInput").ap()
    out = nc.dram_tensor("out_part", [ROWS_OUT, C], F32,
                         kind="ExternalOutput").ap()

    with tile.TileContext(nc) as tc, \
         tc.tile_pool(name="persist", bufs=1) as persist, \
         tc.tile_pool(name="att_sb", bufs=4) as att_sb, \
         tc.tile_pool(name="div_sb", bufs=3) as div_sb, \
         tc.tile_pool(name="div_dram", bufs=3, space="DRAM") as div_dram, \
         tc.tile_pool(name="a2a_dram", bufs=1, space="DRAM") as a2a_dram, \
         tc.tile_pool(name="proj_sb", bufs=4) as proj_sb:

        def ptile(shape, dtype, name):
            return persist.tile(shape, dtype, name=name, tag=name)

        # ---------- persistent SBUF ----------
        xbT = ptile([P, CCHUNKS, T], BF16, name="xbT")
        wqa_s = ptile([P, CCHUNKS, P], BF16, name="wqa_s")
        wka_s = ptile([P, CCHUNKS, P], BF16, name="wka_s")
        wqb_s = ptile([P, CCHUNKS, D], BF16, name="wqb_s")
        wkb_s = ptile([P, CCHUNKS, D], BF16, name="wkb_s")
        wva_s = ptile([P, CCHUNKS, JC], BF16, name="wva_s")
        wpt_s = ptile([P, 7, C], BF16, name="wpt_s")
        trimask = ptile([P, P], BF16, name="trimask")
        m2mask = ptile([P, 2 * P], BF16, name="m2mask")
        bqa_t = ptile([P, 1], F32, name="bqa_t")
        bqb_t = ptile([D, 1], F32, name="bqb_t")
        bka_t = ptile([P, 1], F32, name="bka_t")
        bkb_t = ptile([D, 1], F32, name="bkb_t")
        bv_bc = ptile([P, JC], F32, name="bv_bc")

        # Q^T/K^T in dual-fp8 256-slot layout: slot s = 64*h + d at
        # (i = s//128, p = s%128).  qtf shared; ktf zero-padded per head.
        qtf = ptile([P, 2, T], F8, name="qtf")
        ktf = [ptile([P, 2, T], F8, name=f"ktf{h}") for h in range(HPC)]
        vones = ptile([P, RCHUNKS, HPC, D + 1], BF16, name="vones")
        # agT per head: [p = 64*half + d, b2, sp, q]; plus a bias tile
        agTh = [ptile([P, 2, 2, QCW], BF16, name=f"agT{h}")
                for h in range(HPC)]
        agTb = ptile([P, 2, QCW], BF16, name="agTb")

        # input loads: all on the scalar HWDGE queue, ordered by first use;
        # xbt column blocks are interleaved so QKV can start early
        def load_xbt(qc):
            cs = slice(qc * QCW, (qc + 1) * QCW)
            nc.scalar.dma_start(xbT[:, :, cs], xbt[:, :, cs])

        nc.scalar.dma_start(wqa_s, wqa)
        nc.scalar.dma_start(wqb_s, wqb)
        load_xbt(0)
        nc.sync.dma_start(wka_s, wka)
        nc.sync.dma_start(wkb_s, wkb)
        nc.sync.dma_start(bka_t, bka[:, None])
        nc.sync.dma_start(bkb_t, bkb[:, None])
        nc.sync.dma_start(bqa_t, bqa[:, None])
        nc.sync.dma_start(bqb_t, bqb[:, None])
        nc.sync.dma_start(bv_bc, bv[None, :].to_broadcast((P, JC)))
        nc.sync.dma_start(trimask, tri)
        nc.sync.dma_start(m2mask, mask2)
        load_xbt(1)
        nc.scalar.dma_start(wva_s, wva)
        for qc in range(2, NQC):
            load_xbt(qc)

        # zero the dead fp8 slots one column-quarter at a time; quarter q
        # is emitted just before the first attention block that reads it so
        # the Pool mask muls are not stuck behind a long memset queue
        def memset_quarter(colq):
            cs = slice(colq * (T // 4), (colq + 1) * (T // 4))
            nc.gpsimd.memset(ktf[0][D:P, 0, cs], 0.0)
            nc.gpsimd.memset(ktf[0][:, 1, cs], 0.0)
            nc.gpsimd.memset(ktf[1][0:D, 0, cs], 0.0)
            nc.gpsimd.memset(ktf[1][:, 1, cs], 0.0)
            nc.gpsimd.memset(ktf[2][:, 0, cs], 0.0)
            nc.gpsimd.memset(ktf[2][D:P, 1, cs], 0.0)
            nc.gpsimd.memset(qtf[D:P, 1, cs], 0.0)

        memset_quarter(0)
        nc.gpsimd.memset(vones[:, :, :, D:D + 1], 1.0)
        nc.gpsimd.memset(agTb, 0.0)
        nc.gpsimd.memset(agTb[0:1, :, :], 1.0)

        # pre-warm the ACT exp table
        warm = div_sb.tile([P, 1], F32, name="warm", tag="warm")
        nc.scalar.activation(warm, bqa_t, mybir.ActivationFunctionType.Exp)

        a2a_in = [a2a_dram.tile([NCORES, D, QCW], BF16, name=f"a2a_in{h}",
                                tag=f"a2a_in{h}") for h in range(HPC)]
        a2a_out = [a2a_dram.tile([NCORES * D * QCW, 1], BF16,
                                 name=f"a2a_out{h}", tag=f"a2a_out{h}")
                   for h in range(HPC)]

        # two PSUM pools: S tiles and PV accumulators; released separately
        # so the projection can take over the S banks at the last exp
        ps = tc.alloc_tile_pool(name="ps", bufs=2, space="PSUM")

        # ---- QKV micro-units, injected into attention kc-slots ----
        def qa_unit(qc):
            cs = slice(qc * QCW, (qc + 1) * QCW)

            def emit():
                t = ps.tile([P, QGW], F32, name="tu", tag="pss")
                psq = t[:, 0:QCW]
                for cc in range(CCHUNKS):
                    nc.tensor.matmul(psq, wqa_s[:, cc, :], xbT[:, cc, cs],
                                     start=(cc == 0), stop=(cc == CCHUNKS - 1))
                nc.vector.tensor_scalar_add(qtf[:, 0, cs], psq, bqa_t)
            return emit

        def qb_unit(qc):
            cs = slice(qc * QCW, (qc + 1) * QCW)

            def emit():
                t = ps.tile([P, QGW], F32, name="tu", tag="pss")
                psq = t[0:D, 0:QCW]
                for cc in range(CCHUNKS):
                    nc.tensor.matmul(psq, wqb_s[:, cc, :], xbT[:, cc, cs],
                                     start=(cc == 0), stop=(cc == CCHUNKS - 1))
                nc.vector.tensor_scalar_add(qtf[0:D, 1, cs], psq, bqb_t)
            return emit

        def kb_unit(qc):
            # head-2's K tail: only needed once (2, *) blocks run
            cs = slice(qc * QCW, (qc + 1) * QCW)

            def emit():
                t = ps.tile([P, QGW], F32, name="tu", tag="pss")
                psk = t[0:D, 0:QCW]
                for cc in range(CCHUNKS):
                    nc.tensor.matmul(psk, wkb_s[:, cc, :], xbT[:, cc, cs],
                                     start=(cc == 0), stop=(cc == CCHUNKS - 1))
                nc.vector.tensor_scalar_add(ktf[2][0:D, 1, cs], psk, bkb_t)
            return emit

        def ka_unit(qc):
            cs = slice(qc * QCW, (qc + 1) * QCW)

            def emit():
                t = ps.tile([P, QGW], F32, name="tu", tag="pss")
                psk = t[:, 0:QCW]
                for cc in range(CCHUNKS):
                    nc.tensor.matmul(psk, wka_s[:, cc, :], xbT[:, cc, cs],
                                     start=(cc == 0),
                                     stop=(cc == CCHUNKS - 1))
                nc.vector.tensor_scalar_add(ktf[0][0:D, 0, cs],
                                            psk[0:D, :], bka_t[0:D])
                nc.vector.tensor_scalar_add(ktf[1][D:P, 0, cs],
                                            psk[D:P, :], bka_t[D:P])
            return emit

        def v_unit(rc):
            def emit():
                t = ps.tile([P, QGW], F32, name="tu", tag="pss")
                psv = t[:, 0:JC]
                for cc in range(CCHUNKS):
                    nc.tensor.matmul(psv, xbT[:, cc, rc * P:(rc + 1) * P],
                                     wva_s[:, cc, :], start=(cc == 0),
                                     stop=(cc == CCHUNKS - 1))
                nc.vector.tensor_add(
                    vones[:, rc, :, 0:D],
                    psv.rearrange("p (h d) -> p h d", h=HPC),
                    bv_bc.rearrange("p (h d) -> p h d", h=HPC))
            return emit

        class Feeder:
            """Deadline-sorted QKV unit queue; advance() emits overdue units
            plus at most ~one slot's worth of PE slack (budget in ns) of
            soon-due units, so PE bursts never starve ACT."""

            def __init__(self):
                self.units = []  # (deadline_slot, cost_ns, emit_fn)
                self.i = 0

            def seal(self):
                self.units.sort(key=lambda u: u[0])

            def advance(self, slot, budget=800.0, horizon=12):
                spent = 0.0
                while self.i < len(self.units):
                    dl, cost, emit = self.units[self.i]
                    if dl >= slot and (dl > slot + horizon or spent >= budget):
                        break
                    emit()
                    spent += cost
                    self.i += 1

            def flush(self):
                while self.i < len(self.units):
                    self.units[self.i][2]()
                    self.i += 1

        feeder = Feeder()

        pending_div = [None]

        def flush_div():
            if pending_div[0] is not None:
                pending_div[0]()
                pending_div[0] = None

        def emit_att(h, qg, slot0=None):
            pso = ps.tile([D + 1, QGW], F32, name="pso", tag="pso")
            nkc = (qg + 1) * KPG
            diag0 = qg * KPG

            def emit_pv(kc_e, qoff, pT):
                for half in range(2):
                    kc = kc_e + half
                    for sub in range(QGW // QCW):
                        lo, hi = max(qoff, sub * QCW), (sub + 1) * QCW
                        if lo >= hi:
                            continue
                        nc.tensor.matmul(
                            pso[:, lo:hi], vones[:, kc, h, :],
                            pT[:, half, lo:hi],
                            start=(kc == 0), stop=(kc == nkc - 1))

            pending = None  # PV is emitted one pair late so PE is never
            for kcp in range(nkc // 2):  # stuck waiting on the Pool masks
                kc_e = 2 * kcp
                if kcp == 2:
                    flush_div()  # prev block's atile mul: its DRAM-bounced
                    # broadcast has landed by now, so DVE does not stall
                if slot0 is not None:
                    feeder.advance(slot0 + kc_e)
                qoff = max(0, (kc_e - diag0) * P)
                pT = att_sb.tile([P, 2, QGW], BF16, name="pT", tag="pT")
                for half in range(2):
                    kc = kc_e + half
                    pss = ps.tile([P, QGW], F32, name="pss", tag="pss")
                    for sub in range(QGW // QCW):
                        lo, hi = max(qoff, sub * QCW), (sub + 1) * QCW
                        if lo >= hi:
                            continue
                        nc.tensor.matmul(
                            pss[:, lo:hi], ktf[h][:, :, kc * P:(kc + 1) * P],
                            qtf[:, :, qg * QGW + lo:qg * QGW + hi],
                            start=True, stop=True, perf_mode=DR)
                    nc.scalar.activation(
                        pT[:, half, qoff:QGW], pss[:, qoff:QGW],
                        mybir.ActivationFunctionType.Exp, scale=SCALE)
                if kc_e >= diag0:
                    # causal trim: even chunk triangle; odd chunk
                    # [zeros | triangle] over 256 cols
                    nc.gpsimd.tensor_mul(pT[:, 0, qoff:qoff + P],
                                         pT[:, 0, qoff:qoff + P], trimask)
                    nc.gpsimd.tensor_mul(pT[:, 1, qoff:qoff + 2 * P],
                                         pT[:, 1, qoff:qoff + 2 * P], m2mask)
                if pending is not None:
                    emit_pv(*pending)
                pending = (kc_e, qoff, pT)
            emit_pv(*pending)
            recip = div_sb.tile([1, QGW], F32, name="recip", tag="recip")
            nc.vector.reciprocal(recip, pso[D:D + 1, :])
            rdram = div_dram.tile([1, QGW], F32, name="rdram", tag="rdram")
            nc.sync.dma_start(rdram, recip)
            rbc = div_sb.tile([D, QGW], F32, name="rbc", tag="rbc")
            nc.sync.dma_start(rbc, rdram.to_broadcast((D, QGW)))

            def div_mul(h=h, qg=qg, pso=pso, rbc=rbc):
                atile = div_sb.tile([D, QGW], BF16, name="atile", tag="atile")
                nc.vector.tensor_mul(atile, pso[0:D, :], rbc)
                for half in range(2):
                    nc.sync.dma_start(
                        a2a_in[h][2 * qg + half, :, :],
                        atile[:, half * QCW:(half + 1) * QCW])
            pending_div[0] = div_mul

        def emit_a2a(h):
            nc.gpsimd.collective_compute(
                "AllToAll", mybir.AluOpType.bypass,
                replica_groups=[list(range(NCORES))],
                ins=[a2a_in[h].opt()], outs=[a2a_out[h].opt()])
            for b2 in range(2):
                for sp in range(2):
                    s0 = 4 * b2 + 2 * sp
                    nc.sync.dma_start(
                        agTh[h][:, b2, sp, :],
                        a2a_out[h][s0 * D * QCW:(s0 + 2) * D * QCW, 0]
                        .rearrange("(p q) -> p q", q=QCW))

        # ---- schedule: prefix QKV for the first query group, then inject
        # the remaining QKV micro-units into attention kc-slots just before
        # their deadline, so PE fills ACT-bound attention slack ----
        # PE p-state warm-up: ~4us of throwaway matmuls that only depend on
        # the first weight load, so the real prefix is costed at full clock
        jt = ps.tile([P, QGW], F32, name="jt", tag="pss")
        for _ in range(24):
            nc.tensor.matmul(jt[:, 0:P], wqa_s[:, 0, :], wqa_s[:, 0, :],
                             start=True, stop=True)

        for qc in (0, 1):
            qa_unit(qc)()
            qb_unit(qc)()
            ka_unit(qc)()
        v_unit(0)()
        v_unit(1)()

        # block order: interleave heads so head-0's QKV deadlines spread over
        # many slots; two head-2 blocks run before (0,3) for extra slack
        order = [(0, 0), (1, 0), (0, 1), (1, 1), (0, 2), (1, 2), (2, 0),
                 (2, 1), (0, 3), "a2a0", (1, 3), "a2a1", (2, 2), (2, 3),
                 "a2a2"]
        start_slot = {}
        s = 0
        for blk in order:
            if isinstance(blk, tuple):
                start_slot[blk] = s
                s += (blk[1] + 1) * KPG
        QK_NS, V_NS = 1278.0, 480.0
        for qc in range(2, NQC):
            # stagger the Q units backward from the block-start deadline
            dl = start_slot[(0, qc // 2)] - 1
            feeder.units.append((dl - 6 * (1 - qc % 2) - 3, QK_NS,
                                 qa_unit(qc)))
            feeder.units.append((dl - 6 * (1 - qc % 2), QK_NS,
                                 qb_unit(qc)))
            feeder.units.append((start_slot[(0, qc // 2)] + 4 * qc - 1, QK_NS,
                                 ka_unit(qc)))
        for qc in range(NQC):
            feeder.units.append((start_slot[(2, qc // 2)] + 4 * qc - 1, QK_NS,
                                 kb_unit(qc)))
        for rc in range(2, RCHUNKS):
            feeder.units.append((start_slot[(0, rc // KPG)] + rc - 1, V_NS,
                                 v_unit(rc)))
        feeder.seal()

        for blk in order:
            if blk == "a2a0":
                flush_div()
                emit_a2a(0)
            elif blk == "a2a1":
                flush_div()
                emit_a2a(1)
            elif blk == "a2a2":
                flush_div()
                emit_a2a(2)
            else:
                h, qg = blk
                if blk[0] == 0 and blk[1] >= 1:
                    memset_quarter(blk[1])
                feeder.advance(start_slot[blk] - 1)
                emit_att(h, qg, slot0=start_slot[blk])
        feeder.flush()
        ps.release()

        # wpt is only needed by the projection; load it out of the hot path
        nc.scalar.dma_start(wpt_s, wpt)

        # ---------- output projection (bias via ones-row chunk 6) ----------
        ps_pj = tc.alloc_tile_pool(name="ps_pj", bufs=2, space="PSUM")
        tiles = [(b2, rc) for b2 in range(2) for rc in range(QCW // P)]

        # short PE ramp-up before pass A (it dispatches into an idle PE)
        jt1 = ps_pj.tile([P, C], F32, name="jt1", tag="psj")
        for _ in range(20):
            nc.tensor.matmul(jt1[:, 0:P], wpt_s[:, 0, 0:P], wpt_s[:, 0, 0:P],
                             start=True, stop=True)

        # pass A: bias + heads 0/1 chunks -> bf16 partials, overlapped with
        # the last division chain and the final AllToAll
        partials = []
        for t_i, (b2, rc) in enumerate(tiles):
            psj = ps_pj.tile([P, C], F32, name="psjA", tag="psj")
            qs = slice(rc * P, (rc + 1) * P)
            for ki, k in enumerate([6, 0, 1, 2, 3]):
                st, sp = (ki == 0), (ki == 4)
                lhsT = (agTb[:, b2, qs] if k == 6
                        else agTh[k // 2][:, b2, k % 2, qs])
                for lo, hi in ((0, QCW), (QCW, C)):
                    nc.tensor.matmul(psj[:, lo:hi], lhsT,
                                     wpt_s[:, k, lo:hi], start=st, stop=sp)
            part = persist.tile([P, C], BF16, name=f"part{t_i}",
                                tag=f"part{t_i}")
            if t_i % 2:
                nc.scalar.copy(part, psj)
            else:
                nc.vector.tensor_copy(part, psj)
            partials.append(part)

        # keep PE's p-state hot through the collective flight
        jt2 = ps_pj.tile([P, C], F32, name="jt2", tag="psj")
        for _ in range(100):
            nc.tensor.matmul(jt2[:, 0:QCW], wpt_s[:, 0, 0:P],
                             wpt_s[:, 0, 0:QCW], start=True, stop=True)

        # pass B: head-2 chunks + partial + store
        for t_i, (b2, rc) in enumerate(tiles):
            psj = ps_pj.tile([P, C], F32, name="psjB", tag="psj")
            qs = slice(rc * P, (rc + 1) * P)
            for ki, k in enumerate([4, 5]):
                st, sp = (ki == 0), (ki == 1)
                lhsT = agTh[2][:, b2, k % 2, qs]
                for lo, hi in ((0, QCW), (QCW, C)):
                    nc.tensor.matmul(psj[:, lo:hi], lhsT,
                                     wpt_s[:, k, lo:hi], start=st, stop=sp)
            osb = proj_sb.tile([P, C], F32, name="osb", tag="osb")
            nc.vector.tensor_add(osb, psj, partials[t_i])
            row0 = b2 * QCW + rc * P
            eng = nc.sync if t_i % 2 else nc.scalar
            eng.dma_start(out[row0:row0 + P, :], osb)
        ps_pj.release()

    nc.compile()
    return nc


def _prep_core_inputs(x, Wq, Wk, Wv, Wp, bq, bk, bv, bp):
    """Host-side transposes/folds shared across cores, then per-core dicts."""
    xbt = []
    for b in range(B):
        xt = x[b].T.reshape(CCHUNKS, P, T).transpose(1, 0, 2)
        xbt.append(np.ascontiguousarray(xt.astype(BF16NP)))

    def fold_w(w):  # w [features, C] -> [128, CCHUNKS, features]
        wt = w.T.reshape(CCHUNKS, P, w.shape[0]).transpose(1, 0, 2)
        return np.ascontiguousarray(wt.astype(BF16NP))

    tri_np = np.triu(np.ones((P, P), dtype=np.float32)).astype(BF16NP)
    mask2_np = np.concatenate(
        [np.zeros((P, P), dtype=np.float32),
         np.triu(np.ones((P, P), dtype=np.float32))], axis=1).astype(BF16NP)

    in_maps = []
    for core in range(NCORES):
        b, hg = core // GROUPS, core % GROUPS
        js = slice(JC * hg, JC * (hg + 1))
        wq_c, wk_c, wv_c = Wq[js], Wk[js], Wv[js]
        # wpt: [p = 64*half + d, k, c]; k = 2*h_local + sp ->
        #   global head g = 3*(2*sp+half) + h_local (within the batch);
        #   chunk 6 row 0 = bp.
        wpt = np.zeros((P, 7, C), dtype=np.float32)
        for k in range(6):
            h_local, sp = k // 2, k % 2
            for half in range(2):
                g = HPC * (2 * sp + half) + h_local
                wpt[half * D:(half + 1) * D, k, :] = Wp[:, D * g:D * (g + 1)].T
        wpt[0, 6, :] = bp
        in_maps.append({
            "xbt": xbt[b],
            "wqa": fold_w(wq_c[0:P]),
            "wka": fold_w(wk_c[0:P]),
            "wqb": fold_w(wq_c[P:JC]), "wkb": fold_w(wk_c[P:JC]),
            "wva": fold_w(wv_c),
            "wpt": np.ascontiguousarray(wpt.astype(BF16NP)),
            "bqa": np.ascontiguousarray(bq[js][0:P]),
            "bqb": np.ascontiguousarray(bq[js][P:JC]),
            "bka": np.ascontiguousarray(bk[js][0:P]),
            "bkb": np.ascontiguousarray(bk[js][P:JC]),
            "bv": np.ascontiguousarray(bv[js]),
            "tri": tri_np, "mask2": mask2_np,
        })
    return in_maps


def kernel(**inputs) -> np.ndarray:
    global LAST_RESULTS
    f32 = lambda k: np.ascontiguousarray(np.asarray(inputs[k], dtype=np.float32))
    x, Wq, Wk, Wv, Wp = f32("x"), f32("Wq"), f32("Wk"), f32("Wv"), f32("Wp")
    bq, bk, bv, bp = f32("bq"), f32("bk"), f32("bv"), f32("bp")

    if "nc" not in _CACHE:
        _CACHE["nc"] = _build()
    nc = _CACHE["nc"]

    in_maps = _prep_core_inputs(x, Wq, Wk, Wv, Wp, bq, bk, bv, bp)
    res = run_bass_kernel_spmd(nc, in_maps, core_ids=list(range(NCORES)))
    LAST_RESULTS = res

    out = np.empty((B, T, C), dtype=np.float32)
    for core in range(NCORES):
        part = res.results[core]["out_part"]
        out[0, core * QCW:(core + 1) * QCW, :] = part[:QCW]
        out[1, core * QCW:(core + 1) * QCW, :] = part[QCW:]
    return out


# revision 25
# speedup vs baseline: 1.0435x; 1.0222x over previous
# Causal self-attention kernel for 8 Trainium2 NeuronCores (Bass/Tile).
#
# Problem: x:(2,4096,768) f32, 12 heads, head_dim 64, causal mask, torch-Linear
# Q/K/V/out projections. out = softmax(QK^T/8, causal) V @ Wp^T + biases.
#
# Sharding: core i = batch i//4, head group i%4 (3 heads). All transposes,
# weight folds and dtype casts are done host-side in numpy; the device sees
# contraction-major bf16 operands and DMAs them straight into SBUF.
#
# Device pipeline per core (emission is hand-interleaved so QKV projection
# chunks fill PE slack between attention query-groups, and one shared PSUM
# pool keeps buffer reuse in timeline order):
#   QKV: bf16 projections; Q^T/K^T are cast to fp8 into a 256-slot layout
#     (3 heads x 64 d; K zero-padded per head) for dual-fp8 S matmuls.
#   Attention per head / 1024-query group / 128-key chunk pair:
#     S^T = K_h Q^T as dual-fp8 DoubleRow matmuls (2x PE throughput),
#     P^T = exp(S^T/8) on ACT (bf16 out), causal trim via gpsimd triangle
#     masks, PSUM-accumulate [V|1]^T P^T, divide by the ones-row sum
#     (DVE recip + DRAM-bounce broadcast + multiply).
#   One AllToAll per head re-shards A^T to query-column split, overlapped
#   with the remaining heads' attention.
#   Proj: 8 row tiles x 7 chunks (chunk 6 = ones-row bias, so no bias add);
#   PSUM->SBUF copies alternate ACT/DVE; out DMA alternates HWDGE queues.
import numpy as np
import ml_dtypes

import concourse.bass as bass  # noqa: F401
import concourse.mybir as mybir
import concourse.tile as tile
from concourse import bacc
from concourse.bass_utils import run_bass_kernel_spmd

F32 = mybir.dt.float32
BF16 = mybir.dt.bfloat16
F8 = mybir.dt.float8e4
DR = mybir.MatmulPerfMode.DoubleRow
BF16NP = ml_dtypes.bfloat16

B, T, C, H, D = 2, 4096, 768, 12, 64
NCORES = 8
GROUPS = 4              # cores per batch
HPC = H // GROUPS       # 3 heads per core
JC = HPC * D            # 192 projected features per core
P = 128
CCHUNKS = 6             # contraction chunks of C
RCHUNKS = T // P        # 32
QCW = 512               # psum bank width (f32)
NQC = T // QCW          # 8
QGW = 1024              # attention query-group width
NQG = T // QGW          # 4
KPG = QGW // P          # key chunks per query-group span (8)
ROWS_OUT = T // GROUPS  # 1024 output rows per core
SCALE = 1.0 / 8.0

_CACHE: dict = {}
LAST_RESULTS = None


def _build():
    nc = bacc.Bacc("TRN2", target_bir_lowering=False, debug=False,
                   num_devices=NCORES)

    xbt = nc.dram_tensor("xbt", [P, CCHUNKS, T], BF16, kind="ExternalInput").ap()
    wqa = nc.dram_tensor("wqa", [P, CCHUNKS, P], BF16, kind="ExternalInput").ap()
    wka = nc.dram_tensor("wka", [P, CCHUNKS, P], BF16, kind="ExternalInput").ap()
    wqb = nc.dram_tensor("wqb", [P, CCHUNKS, D], BF16, kind="ExternalInput").ap()
    wkb = nc.dram_tensor("wkb", [P, CCHUNKS, D], BF16, kind="ExternalInput").ap()
    wva = nc.dram_tensor("wva", [P, CCHUNKS, JC], BF16, kind="ExternalInput").ap()
    wpt = nc.dram_tensor("wpt", [P, 7, C], BF16, kind="ExternalInput").ap()
    bqa = nc.dram_tensor("bqa", [P], F32, kind="ExternalInput").ap()
    bqb = nc.dram_tensor("bqb", [D], F32, kind="ExternalInput").ap()
    bka = nc.dram_tensor("bka", [P], F32, kind="ExternalInput").ap()
    bkb = nc.dram_tensor("bkb", [D], F32, kind="ExternalInput").ap()
    bv = nc.dram_tensor("bv", [JC], F32, kind="ExternalInput").ap()
    tri = nc.dram_tensor("tri", [P, P], BF16, kind="ExternalInput").ap()
    mask2 = nc.dram_tensor("mask2", [P, 2 * P], BF16, kind="ExternalInput").ap()
    out = nc.dram_tensor("out_part", [ROWS_OUT, C], F32,
                         kind="ExternalOutput").ap()

    with tile.TileContext(nc) as tc, \
         tc.tile_pool(name="persist", bufs=1) as persist, \
         tc.tile_pool(name="att_sb", bufs=4) as att_sb, \
         tc.tile_pool(name="div_sb", bufs=3) as div_sb, \
         tc.tile_pool(name="div_dram", bufs=3, space="DRAM") as div_dram, \
         tc.tile_pool(name="a2a_dram", bufs=1, space="DRAM") as a2a_dram, \
         tc.tile_pool(name="proj_sb", bufs=4) as proj_sb:

        def ptile(shape, dtype, name):
            return persist.tile(shape, dtype, name=name, tag=name)

        # ---------- persistent SBUF ----------
        xbT = ptile([P, CCHUNKS, T], BF16, name="xbT")
        wqa_s = ptile([P, CCHUNKS, P], BF16, name="wqa_s")
        wka_s = ptile([P, CCHUNKS, P], BF16, name="wka_s")
        wqb_s = ptile([P, CCHUNKS, D], BF16, name="wqb_s")
        wkb_s = ptile([P, CCHUNKS, D], BF16, name="wkb_s")
        wva_s = ptile([P, CCHUNKS, JC], BF16, name="wva_s")
        wpt_s = ptile([P, 7, C], BF16, name="wpt_s")
        trimask = ptile([P, P], BF16, name="trimask")
        m2mask = ptile([P, 2 * P], BF16, name="m2mask")
        bqa_t = ptile([P, 1], F32, name="bqa_t")
        bqb_t = ptile([D, 1], F32, name="bqb_t")
        bka_t = ptile([P, 1], F32, name="bka_t")
        bkb_t = ptile([D, 1], F32, name="bkb_t")
        bv_bc = ptile([P, JC], F32, name="bv_bc")

        # Q^T/K^T in dual-fp8 256-slot layout: slot s = 64*h + d at
        # (i = s//128, p = s%128).  qtf shared; ktf zero-padded per head.
        qtf = ptile([P, 2, T], F8, name="qtf")
        ktf = [ptile([P, 2, T], F8, name=f"ktf{h}") for h in range(HPC)]
        vones = ptile([P, RCHUNKS, HPC, D + 1], BF16, name="vones")
        # agT per head: [p = 64*half + d, b2, sp, q]; plus a bias tile
        agTh = [ptile([P, 2, 2, QCW], BF16, name=f"agT{h}")
                for h in range(HPC)]
        agTb = ptile([P, 2, QCW], BF16, name="agTb")

        # input loads: all on the scalar HWDGE queue, ordered by first use;
        # xbt column blocks are interleaved so QKV can start early
        def load_xbt(qc):
            cs = slice(qc * QCW, (qc + 1) * QCW)
            nc.scalar.dma_start(xbT[:, :, cs], xbt[:, :, cs])

        nc.scalar.dma_start(wqa_s, wqa)
        nc.scalar.dma_start(wqb_s, wqb)
        load_xbt(0)
        nc.sync.dma_start(wka_s, wka)
        nc.sync.dma_start(wkb_s, wkb)
        nc.sync.dma_start(bka_t, bka[:, None])
        nc.sync.dma_start(bkb_t, bkb[:, None])
        nc.sync.dma_start(bqa_t, bqa[:, None])
        nc.sync.dma_start(bqb_t, bqb[:, None])
        nc.sync.dma_start(bv_bc, bv[None, :].to_broadcast((P, JC)))
        nc.sync.dma_start(trimask, tri)
        nc.sync.dma_start(m2mask, mask2)
        load_xbt(1)
        nc.scalar.dma_start(wva_s, wva)
        for qc in range(2, NQC):
            load_xbt(qc)

        # zero the dead fp8 slots one column-quarter at a time; quarter q
        # is emitted just before the first attention block that reads it so
        # the Pool mask muls are not stuck behind a long memset queue
        def memset_quarter(colq):
            cs = slice(colq * (T // 4), (colq + 1) * (T // 4))
            nc.gpsimd.memset(ktf[0][D:P, 0, cs], 0.0)
            nc.gpsimd.memset(ktf[0][:, 1, cs], 0.0)
            nc.gpsimd.memset(ktf[1][0:D, 0, cs], 0.0)
            nc.gpsimd.memset(ktf[1][:, 1, cs], 0.0)
            nc.gpsimd.memset(ktf[2][:, 0, cs], 0.0)
            nc.gpsimd.memset(ktf[2][D:P, 1, cs], 0.0)
            nc.gpsimd.memset(qtf[D:P, 1, cs], 0.0)

        ones_bc = ptile([1, D], F32, name="ones_bc")
        memset_quarter(0)
        nc.gpsimd.memset(ones_bc, 1.0)
        nc.gpsimd.memset(vones[:, :, :, D:D + 1], 1.0)
        nc.gpsimd.memset(agTb, 0.0)
        nc.gpsimd.memset(agTb[0:1, :, :], 1.0)

        # pre-warm the ACT exp table
        warm = div_sb.tile([P, 1], F32, name="warm", tag="warm")
        nc.scalar.activation(warm, bqa_t, mybir.ActivationFunctionType.Exp)

        a2a_in = [a2a_dram.tile([NCORES, D, QCW], BF16, name=f"a2a_in{h}",
                                tag=f"a2a_in{h}") for h in range(HPC)]
        a2a_out = [a2a_dram.tile([NCORES * D * QCW, 1], BF16,
                                 name=f"a2a_out{h}", tag=f"a2a_out{h}")
                   for h in range(HPC)]

        # two PSUM pools: S tiles and PV accumulators; released separately
        # so the projection can take over the S banks at the last exp
        ps = tc.alloc_tile_pool(name="ps", bufs=2, space="PSUM")

        # ---- QKV micro-units, injected into attention kc-slots ----
        def qa_unit(qc):
            cs = slice(qc * QCW, (qc + 1) * QCW)

            def emit():
                t = ps.tile([P, QGW], F32, name="tu", tag="pss")
                psq = t[:, 0:QCW]
                for cc in range(CCHUNKS):
                    nc.tensor.matmul(psq, wqa_s[:, cc, :], xbT[:, cc, cs],
                                     start=(cc == 0), stop=(cc == CCHUNKS - 1))
                nc.vector.tensor_scalar_add(qtf[:, 0, cs], psq, bqa_t)
            return emit

        def qb_unit(qc):
            cs = slice(qc * QCW, (qc + 1) * QCW)

            def emit():
                t = ps.tile([P, QGW], F32, name="tu", tag="pss")
                psq = t[0:D, 0:QCW]
                for cc in range(CCHUNKS):
                    nc.tensor.matmul(psq, wqb_s[:, cc, :], xbT[:, cc, cs],
                                     start=(cc == 0), stop=(cc == CCHUNKS - 1))
                nc.vector.tensor_scalar_add(qtf[0:D, 1, cs], psq, bqb_t)
            return emit

        def kb_unit(qc):
            # head-2's K tail: only needed once (2, *) blocks run
            cs = slice(qc * QCW, (qc + 1) * QCW)

            def emit():
                t = ps.tile([P, QGW], F32, name="tu", tag="pss")
                psk = t[0:D, 0:QCW]
                for cc in range(CCHUNKS):
                    nc.tensor.matmul(psk, wkb_s[:, cc, :], xbT[:, cc, cs],
                                     start=(cc == 0), stop=(cc == CCHUNKS - 1))
                nc.vector.tensor_scalar_add(ktf[2][0:D, 1, cs], psk, bkb_t)
            return emit

        def ka_unit(qc):
            cs = slice(qc * QCW, (qc + 1) * QCW)

            def emit():
                t = ps.tile([P, QGW], F32, name="tu", tag="pss")
                psk = t[:, 0:QCW]
                for cc in range(CCHUNKS):
                    nc.tensor.matmul(psk, wka_s[:, cc, :], xbT[:, cc, cs],
                                     start=(cc == 0),
                                     stop=(cc == CCHUNKS - 1))
                nc.vector.tensor_scalar_add(ktf[0][0:D, 0, cs],
                                            psk[0:D, :], bka_t[0:D])
                nc.vector.tensor_scalar_add(ktf[1][D:P, 0, cs],
                                            psk[D:P, :], bka_t[D:P])
            return emit

        def v_unit(rc):
            def emit():
                t = ps.tile([P, QGW], F32, name="tu", tag="pss")
                psv = t[:, 0:JC]
                for cc in range(CCHUNKS):
                    nc.tensor.matmul(psv, xbT[:, cc, rc * P:(rc + 1) * P],
                                     wva_s[:, cc, :], start=(cc == 0),
                                     stop=(cc == CCHUNKS - 1))
                nc.vector.tensor_add(
                    vones[:, rc, :, 0:D],
                    psv.rearrange("p (h d) -> p h d", h=HPC),
                    bv_bc.rearrange("p (h d) -> p h d", h=HPC))
            return emit

        class Feeder:
            """Deadline-sorted QKV unit queue; advance() emits overdue units
            plus at most ~one slot's worth of PE slack (budget in ns) of
            soon-due units, so PE bursts never starve ACT."""

            def __init__(self):
                self.units = []  # (deadline_slot, cost_ns, emit_fn)
                self.i = 0

            def seal(self):
                self.units.sort(key=lambda u: u[0])

            def advance(self, slot, budget=800.0, horizon=12):
                spent = 0.0
                while self.i < len(self.units):
                    dl, cost, emit = self.units[self.i]
                    if dl >= slot and (dl > slot + horizon or spent >= budget):
                        break
                    emit()
                    spent += cost
                    self.i += 1

            def flush(self):
                while self.i < len(self.units):
                    self.units[self.i][2]()
                    self.i += 1

        feeder = Feeder()

        pending_div = [None]

        def flush_div():
            if pending_div[0] is not None:
                pending_div[0]()
                pending_div[0] = None

        def emit_att(h, qg, slot0=None, last=False):
            pso = ps.tile([D + 1, QGW], F32, name="pso", tag="pso")
            nkc = (qg + 1) * KPG
            diag0 = qg * KPG

            def emit_pv(kc_e, qoff, pT):
                for half in range(2):
                    kc = kc_e + half
                    for sub in range(QGW // QCW):
                        lo, hi = max(qoff, sub * QCW), (sub + 1) * QCW
                        if lo >= hi:
                            continue
                        nc.tensor.matmul(
                            pso[:, lo:hi], vones[:, kc, h, :],
                            pT[:, half, lo:hi],
                            start=(kc == 0), stop=(kc == nkc - 1))

            pending = None  # PV is emitted one pair late so PE is never
            for kcp in range(nkc // 2):  # stuck waiting on the Pool masks
                kc_e = 2 * kcp
                if kcp == 2:
                    flush_div()  # prev block's atile mul: its DRAM-bounced
                    # broadcast has landed by now, so DVE does not stall
                if slot0 is not None:
                    feeder.advance(slot0 + kc_e)
                qoff = max(0, (kc_e - diag0) * P)
                pT = att_sb.tile([P, 2, QGW], BF16, name="pT", tag="pT")
                for half in range(2):
                    kc = kc_e + half
                    pss = ps.tile([P, QGW], F32, name="pss", tag="pss")
                    for sub in range(QGW // QCW):
                        lo, hi = max(qoff, sub * QCW), (sub + 1) * QCW
                        if lo >= hi:
                            continue
                        nc.tensor.matmul(
                            pss[:, lo:hi], ktf[h][:, :, kc * P:(kc + 1) * P],
                            qtf[:, :, qg * QGW + lo:qg * QGW + hi],
                            start=True, stop=True, perf_mode=DR)
                    nc.scalar.activation(
                        pT[:, half, qoff:QGW], pss[:, qoff:QGW],
                        mybir.ActivationFunctionType.Exp, scale=SCALE)
                if kc_e >= diag0:
                    # causal trim: even chunk triangle; odd chunk
                    # [zeros | triangle] over 256 cols
                    nc.gpsimd.tensor_mul(pT[:, 0, qoff:qoff + P],
                                         pT[:, 0, qoff:qoff + P], trimask)
                    nc.gpsimd.tensor_mul(pT[:, 1, qoff:qoff + 2 * P],
                                         pT[:, 1, qoff:qoff + 2 * P], m2mask)
                if pending is not None:
                    emit_pv(*pending)
                pending = (kc_e, qoff, pT)
            emit_pv(*pending)
            recip = div_sb.tile([1, QGW], F32, name="recip", tag="recip")
            nc.vector.reciprocal(recip, pso[D:D + 1, :])
            if last:
                # latency-critical final division: broadcast the reciprocal
                # across partitions with a ones-column PE matmul instead of
                # the DRAM bounce (saves ~5us on the tail)
                pbc = ps.tile([P, QGW], F32, name="pbc", tag="pss")
                for lo, hi in ((0, QCW), (QCW, QGW)):
                    nc.tensor.matmul(pbc[0:D, lo:hi], ones_bc,
                                     recip[:, lo:hi], start=True, stop=True)
                araw = div_sb.tile([D, QGW], F32, name="araw", tag="rbc")
                nc.scalar.copy(araw, pso[0:D, :])  # ACT, parallel to recip
                atile = div_sb.tile([D, QGW], BF16, name="atile", tag="atile")
                nc.vector.tensor_mul(atile, araw, pbc[0:D, :])
                for half in range(2):
                    nc.sync.dma_start(
                        a2a_in[h][2 * qg + half, :, :],
                        atile[:, half * QCW:(half + 1) * QCW])
                return
            rdram = div_dram.tile([1, QGW], F32, name="rdram", tag="rdram")
            nc.sync.dma_start(rdram, recip)
            rbc = div_sb.tile([D, QGW], F32, name="rbc", tag="rbc")
            nc.sync.dma_start(rbc, rdram.to_broadcast((D, QGW)))

            def div_mul(h=h, qg=qg, pso=pso, rbc=rbc):
                atile = div_sb.tile([D, QGW], BF16, name="atile", tag="atile")
                nc.vector.tensor_mul(atile, pso[0:D, :], rbc)
                for half in range(2):
                    nc.sync.dma_start(
                        a2a_in[h][2 * qg + half, :, :],
                        atile[:, half * QCW:(half + 1) * QCW])
            pending_div[0] = div_mul

        def emit_a2a(h):
            nc.gpsimd.collective_compute(
                "AllToAll", mybir.AluOpType.bypass,
                replica_groups=[list(range(NCORES))],
                ins=[a2a_in[h].opt()], outs=[a2a_out[h].opt()])
            for b2 in range(2):
                for sp in range(2):
                    s0 = 4 * b2 + 2 * sp
                    nc.sync.dma_start(
                        agTh[h][:, b2, sp, :],
                        a2a_out[h][s0 * D * QCW:(s0 + 2) * D * QCW, 0]
                        .rearrange("(p q) -> p q", q=QCW))

        # ---- schedule: prefix QKV for the first query group, then inject
        # the remaining QKV micro-units into attention kc-slots just before
        # their deadline, so PE fills ACT-bound attention slack ----
        # PE p-state warm-up: ~4us of throwaway matmuls that only depend on
        # the first weight load, so the real prefix is costed at full clock
        jt = ps.tile([P, QGW], F32, name="jt", tag="pss")
        for _ in range(24):
            nc.tensor.matmul(jt[:, 0:P], wqa_s[:, 0, :], wqa_s[:, 0, :],
                             start=True, stop=True)

        for qc in (0, 1):
            qa_unit(qc)()
            qb_unit(qc)()
            ka_unit(qc)()
        v_unit(0)()
        v_unit(1)()

        # block order: interleave heads so head-0's QKV deadlines spread over
        # many slots; two head-2 blocks run before (0,3) for extra slack
        order = [(0, 0), (1, 0), (0, 1), (1, 1), (0, 2), (1, 2), (2, 0),
                 (2, 1), (0, 3), "a2a0", (1, 3), "a2a1", (2, 2), (2, 3),
                 "a2a2"]
        start_slot = {}
        s = 0
        for blk in order:
            if isinstance(blk, tuple):
                start_slot[blk] = s
                s += (blk[1] + 1) * KPG
        QK_NS, V_NS = 1278.0, 480.0
        for qc in range(2, NQC):
            # stagger the Q units backward from the block-start deadline
            dl = start_slot[(0, qc // 2)] - 1
            feeder.units.append((dl - 6 * (1 - qc % 2) - 3, QK_NS,
                                 qa_unit(qc)))
            feeder.units.append((dl - 6 * (1 - qc % 2), QK_NS,
                                 qb_unit(qc)))
            feeder.units.append((start_slot[(0, qc // 2)] + 4 * qc - 1, QK_NS,
                                 ka_unit(qc)))
        for qc in range(NQC):
            feeder.units.append((start_slot[(2, qc // 2)] + 4 * qc - 1, QK_NS,
                                 kb_unit(qc)))
        for rc in range(2, RCHUNKS):
            feeder.units.append((start_slot[(0, rc // KPG)] + rc - 1, V_NS,
                                 v_unit(rc)))
        feeder.seal()

        for blk in order:
            if blk == "a2a0":
                flush_div()
                emit_a2a(0)
            elif blk == "a2a1":
                flush_div()
                emit_a2a(1)
            elif blk == "a2a2":
                flush_div()
                emit_a2a(2)
            else:
                h, qg = blk
                if blk[0] == 0 and blk[1] >= 1:
                    memset_quarter(blk[1])
                feeder.advance(start_slot[blk] - 1)
                emit_att(h, qg, slot0=start_slot[blk],
                         last=(blk == order[-2]))
        feeder.flush()
        ps.release()

        # wpt is only needed by the projection; load it out of the hot path
        nc.scalar.dma_start(wpt_s, wpt)

        # ---------- output projection (bias via ones-row chunk 6) ----------
        ps_pj = tc.alloc_tile_pool(name="ps_pj", bufs=2, space="PSUM")
        tiles = [(b2, rc) for b2 in range(2) for rc in range(QCW // P)]

        # short PE ramp-up before pass A (it dispatches into an idle PE)
        jt1 = ps_pj.tile([P, C], F32, name="jt1", tag="psj")
        for _ in range(20):
            nc.tensor.matmul(jt1[:, 0:P], wpt_s[:, 0, 0:P], wpt_s[:, 0, 0:P],
                             start=True, stop=True)

        # pass A: bias + heads 0/1 chunks -> bf16 partials, overlapped with
        # the last division chain and the final AllToAll
        partials = []
        for t_i, (b2, rc) in enumerate(tiles):
            psj = ps_pj.tile([P, C], F32, name="psjA", tag="psj")
            qs = slice(rc * P, (rc + 1) * P)
            for ki, k in enumerate([6, 0, 1, 2, 3]):
                st, sp = (ki == 0), (ki == 4)
                lhsT = (agTb[:, b2, qs] if k == 6
                        else agTh[k // 2][:, b2, k % 2, qs])
                for lo, hi in ((0, QCW), (QCW, C)):
                    nc.tensor.matmul(psj[:, lo:hi], lhsT,
                                     wpt_s[:, k, lo:hi], start=st, stop=sp)
            part = persist.tile([P, C], BF16, name=f"part{t_i}",
                                tag=f"part{t_i}")
            if t_i % 2:
                nc.scalar.copy(part, psj)
            else:
                nc.vector.tensor_copy(part, psj)
            partials.append(part)

        # keep PE's p-state hot through the collective flight
        jt2 = ps_pj.tile([P, C], F32, name="jt2", tag="psj")
        for _ in range(82):
            nc.tensor.matmul(jt2[:, 0:QCW], wpt_s[:, 0, 0:P],
                             wpt_s[:, 0, 0:QCW], start=True, stop=True)

        # pass B: head-2 chunks + partial + store
        for t_i, (b2, rc) in enumerate(tiles):
            psj = ps_pj.tile([P, C], F32, name="psjB", tag="psj")
            qs = slice(rc * P, (rc + 1) * P)
            for ki, k in enumerate([4, 5]):
                st, sp = (ki == 0), (ki == 1)
                lhsT = agTh[2][:, b2, k % 2, qs]
                for lo, hi in ((0, QCW), (QCW, C)):
                    nc.tensor.matmul(psj[:, lo:hi], lhsT,
                                     wpt_s[:, k, lo:hi], start=st, stop=sp)
            osb = proj_sb.tile([P, C], F32, name="osb", tag="osb")
            nc.vector.tensor_add(osb, psj, partials[t_i])
            row0 = b2 * QCW + rc * P
            eng = nc.sync if t_i % 2 else nc.scalar
            eng.dma_start(out[row0:row0 + P, :], osb)
        ps_pj.release()

    nc.compile()
    return nc


def _prep_core_inputs(x, Wq, Wk, Wv, Wp, bq, bk, bv, bp):
    """Host-side transposes/folds shared across cores, then per-core dicts."""
    xbt = []
    for b in range(B):
        xt = x[b].T.reshape(CCHUNKS, P, T).transpose(1, 0, 2)
        xbt.append(np.ascontiguousarray(xt.astype(BF16NP)))

    def fold_w(w):  # w [features, C] -> [128, CCHUNKS, features]
        wt = w.T.reshape(CCHUNKS, P, w.shape[0]).transpose(1, 0, 2)
        return np.ascontiguousarray(wt.astype(BF16NP))

    tri_np = np.triu(np.ones((P, P), dtype=np.float32)).astype(BF16NP)
    mask2_np = np.concatenate(
        [np.zeros((P, P), dtype=np.float32),
         np.triu(np.ones((P, P), dtype=np.float32))], axis=1).astype(BF16NP)

    in_maps = []
    for core in range(NCORES):
        b, hg = core // GROUPS, core % GROUPS
        js = slice(JC * hg, JC * (hg + 1))
        wq_c, wk_c, wv_c = Wq[js], Wk[js], Wv[js]
        # wpt: [p = 64*half + d, k, c]; k = 2*h_local + sp ->
        #   global head g = 3*(2*sp+half) + h_local (within the batch);
        #   chunk 6 row 0 = bp.
        wpt = np.zeros((P, 7, C), dtype=np.float32)
        for k in range(6):
            h_local, sp = k // 2, k % 2
            for half in range(2):
                g = HPC * (2 * sp + half) + h_local
                wpt[half * D:(half + 1) * D, k, :] = Wp[:, D * g:D * (g + 1)].T
        wpt[0, 6, :] = bp
        in_maps.append({
            "xbt": xbt[b],
            "wqa": fold_w(wq_c[0:P]),
            "wka": fold_w(wk_c[0:P]),
            "wqb": fold_w(wq_c[P:JC]), "wkb": fold_w(wk_c[P:JC]),
            "wva": fold_w(wv_c),
            "wpt": np.ascontiguousarray(wpt.astype(BF16NP)),
            "bqa": np.ascontiguousarray(bq[js][0:P]),
            "bqb": np.ascontiguousarray(bq[js][P:JC]),
            "bka": np.ascontiguousarray(bk[js][0:P]),
            "bkb": np.ascontiguousarray(bk[js][P:JC]),
            "bv": np.ascontiguousarray(bv[js]),
            "tri": tri_np, "mask2": mask2_np,
        })
    return in_maps


def kernel(**inputs) -> np.ndarray:
    global LAST_RESULTS
    f32 = lambda k: np.ascontiguousarray(np.asarray(inputs[k], dtype=np.float32))
    x, Wq, Wk, Wv, Wp = f32("x"), f32("Wq"), f32("Wk"), f32("Wv"), f32("Wp")
    bq, bk, bv, bp = f32("bq"), f32("bk"), f32("bv"), f32("bp")

    if "nc" not in _CACHE:
        _CACHE["nc"] = _build()
    nc = _CACHE["nc"]

    in_maps = _prep_core_inputs(x, Wq, Wk, Wv, Wp, bq, bk, bv, bp)
    res = run_bass_kernel_spmd(nc, in_maps, core_ids=list(range(NCORES)))
    LAST_RESULTS = res

    out = np.empty((B, T, C), dtype=np.float32)
    for core in range(NCORES):
        part = res.results[core]["out_part"]
        out[0, core * QCW:(core + 1) * QCW, :] = part[:QCW]
        out[1, core * QCW:(core + 1) * QCW, :] = part[QCW:]
    return out


# revision 26
# speedup vs baseline: 1.0676x; 1.0231x over previous
# Causal self-attention kernel for 8 Trainium2 NeuronCores (Bass/Tile).
#
# Problem: x:(2,4096,768) f32, 12 heads, head_dim 64, causal mask, torch-Linear
# Q/K/V/out projections. out = softmax(QK^T/8, causal) V @ Wp^T + biases.
#
# Sharding: core i = batch i//4, head group i%4 (3 heads). All transposes,
# weight folds and dtype casts are done host-side in numpy; the device sees
# contraction-major bf16 operands and DMAs them straight into SBUF.
#
# Device pipeline per core (emission is hand-interleaved so QKV projection
# chunks fill PE slack between attention query-groups, and one shared PSUM
# pool keeps buffer reuse in timeline order):
#   QKV: bf16 projections; Q^T/K^T are cast to fp8 into a 256-slot layout
#     (3 heads x 64 d; K zero-padded per head) for dual-fp8 S matmuls.
#   Attention per head / 1024-query group / 128-key chunk pair:
#     S^T = K_h Q^T as dual-fp8 DoubleRow matmuls (2x PE throughput),
#     P^T = exp(S^T/8) on ACT (bf16 out), causal trim via gpsimd triangle
#     masks, PSUM-accumulate [V|1]^T P^T, divide by the ones-row sum
#     (DVE recip + DRAM-bounce broadcast + multiply).
#   One AllToAll per head re-shards A^T to query-column split, overlapped
#   with the remaining heads' attention.
#   Proj: 8 row tiles x 7 chunks (chunk 6 = ones-row bias, so no bias add);
#   PSUM->SBUF copies alternate ACT/DVE; out DMA alternates HWDGE queues.
import numpy as np
import ml_dtypes

import concourse.bass as bass  # noqa: F401
import concourse.mybir as mybir
import concourse.tile as tile
from concourse import bacc
from concourse.bass_utils import run_bass_kernel_spmd

F32 = mybir.dt.float32
BF16 = mybir.dt.bfloat16
F8 = mybir.dt.float8e4
DR = mybir.MatmulPerfMode.DoubleRow
BF16NP = ml_dtypes.bfloat16

B, T, C, H, D = 2, 4096, 768, 12, 64
NCORES = 8
GROUPS = 4              # cores per batch
HPC = H // GROUPS       # 3 heads per core
JC = HPC * D            # 192 projected features per core
P = 128
CCHUNKS = 6             # contraction chunks of C
RCHUNKS = T // P        # 32
QCW = 512               # psum bank width (f32)
NQC = T // QCW          # 8
QGW = 1024              # attention query-group width
NQG = T // QGW          # 4
KPG = QGW // P          # key chunks per query-group span (8)
ROWS_OUT = T // GROUPS  # 1024 output rows per core
SCALE = 1.0 / 8.0

_CACHE: dict = {}
LAST_RESULTS = None


def _build():
    nc = bacc.Bacc("TRN2", target_bir_lowering=False, debug=False,
                   num_devices=NCORES)

    xbt = nc.dram_tensor("xbt", [P, CCHUNKS, T], BF16, kind="ExternalInput").ap()
    wqa = nc.dram_tensor("wqa", [P, CCHUNKS, P], BF16, kind="ExternalInput").ap()
    wka = nc.dram_tensor("wka", [P, CCHUNKS, P], BF16, kind="ExternalInput").ap()
    wqb = nc.dram_tensor("wqb", [P, CCHUNKS, D], BF16, kind="ExternalInput").ap()
    wkb = nc.dram_tensor("wkb", [P, CCHUNKS, D], BF16, kind="ExternalInput").ap()
    wva = nc.dram_tensor("wva", [P, CCHUNKS, JC], BF16, kind="ExternalInput").ap()
    wpt = nc.dram_tensor("wpt", [P, 7, C], BF16, kind="ExternalInput").ap()
    bqa = nc.dram_tensor("bqa", [P], F32, kind="ExternalInput").ap()
    bqb = nc.dram_tensor("bqb", [D], F32, kind="ExternalInput").ap()
    bka = nc.dram_tensor("bka", [P], F32, kind="ExternalInput").ap()
    bkb = nc.dram_tensor("bkb", [D], F32, kind="ExternalInput").ap()
    bv = nc.dram_tensor("bv", [JC], F32, kind="ExternalInput").ap()
    tri = nc.dram_tensor("tri", [P, P], BF16, kind="ExternalInput").ap()
    mask2 = nc.dram_tensor("mask2", [P, 2 * P], BF16, kind="ExternalInput").ap()
    out = nc.dram_tensor("out_part", [ROWS_OUT, C], F32,
                         kind="ExternalOutput").ap()

    with tile.TileContext(nc) as tc, \
         tc.tile_pool(name="persist", bufs=1) as persist, \
         tc.tile_pool(name="att_sb", bufs=4) as att_sb, \
         tc.tile_pool(name="div_sb", bufs=3) as div_sb, \
         tc.tile_pool(name="div_dram", bufs=3, space="DRAM") as div_dram, \
         tc.tile_pool(name="a2a_dram", bufs=1, space="DRAM") as a2a_dram, \
         tc.tile_pool(name="proj_sb", bufs=4) as proj_sb:

        def ptile(shape, dtype, name):
            return persist.tile(shape, dtype, name=name, tag=name)

        # ---------- persistent SBUF ----------
        xbT = ptile([P, CCHUNKS, T], BF16, name="xbT")
        wqa_s = ptile([P, CCHUNKS, P], BF16, name="wqa_s")
        wka_s = ptile([P, CCHUNKS, P], BF16, name="wka_s")
        wqb_s = ptile([P, CCHUNKS, D], BF16, name="wqb_s")
        wkb_s = ptile([P, CCHUNKS, D], BF16, name="wkb_s")
        wva_s = ptile([P, CCHUNKS, JC], BF16, name="wva_s")
        wpt_s = ptile([P, 7, C], BF16, name="wpt_s")
        trimask = ptile([P, P], BF16, name="trimask")
        m2mask = ptile([P, 2 * P], BF16, name="m2mask")
        bqa_t = ptile([P, 1], F32, name="bqa_t")
        bqb_t = ptile([D, 1], F32, name="bqb_t")
        bka_t = ptile([P, 1], F32, name="bka_t")
        bkb_t = ptile([D, 1], F32, name="bkb_t")
        bv_bc = ptile([P, JC], F32, name="bv_bc")

        # Q^T/K^T in dual-fp8 256-slot layout: slot s = 64*h + d at
        # (i = s//128, p = s%128).  qtf shared; ktf zero-padded per head.
        qtf = ptile([P, 2, T], F8, name="qtf")
        ktf = [ptile([P, 2, T], F8, name=f"ktf{h}") for h in range(HPC)]
        vones = ptile([P, RCHUNKS, HPC, D + 1], BF16, name="vones")
        # agT per head: [p = 64*half + d, b2, sp, q]; plus a bias tile
        agTh = [ptile([P, 2, 2, QCW], BF16, name=f"agT{h}")
                for h in range(HPC)]
        agTb = ptile([P, 2, QCW], BF16, name="agTb")

        # input loads: all on the scalar HWDGE queue, ordered by first use;
        # xbt column blocks are interleaved so QKV can start early
        def load_xbt(qc):
            cs = slice(qc * QCW, (qc + 1) * QCW)
            nc.scalar.dma_start(xbT[:, :, cs], xbt[:, :, cs])

        nc.scalar.dma_start(wqa_s, wqa)
        nc.scalar.dma_start(wqb_s, wqb)
        load_xbt(0)
        nc.sync.dma_start(wka_s, wka)
        nc.sync.dma_start(wkb_s, wkb)
        nc.sync.dma_start(bka_t, bka[:, None])
        nc.sync.dma_start(bkb_t, bkb[:, None])
        nc.sync.dma_start(bqa_t, bqa[:, None])
        nc.sync.dma_start(bqb_t, bqb[:, None])
        nc.sync.dma_start(bv_bc, bv[None, :].to_broadcast((P, JC)))
        nc.sync.dma_start(trimask, tri)
        nc.sync.dma_start(m2mask, mask2)
        load_xbt(1)
        nc.scalar.dma_start(wva_s, wva)
        for qc in range(2, NQC):
            load_xbt(qc)

        # zero the dead fp8 slots one column-quarter at a time; quarter q
        # is emitted just before the first attention block that reads it so
        # the Pool mask muls are not stuck behind a long memset queue
        def memset_quarter(colq):
            cs = slice(colq * (T // 4), (colq + 1) * (T // 4))
            nc.gpsimd.memset(ktf[0][D:P, 0, cs], 0.0)
            nc.gpsimd.memset(ktf[0][:, 1, cs], 0.0)
            nc.gpsimd.memset(ktf[1][0:D, 0, cs], 0.0)
            nc.gpsimd.memset(ktf[1][:, 1, cs], 0.0)
            nc.gpsimd.memset(ktf[2][:, 0, cs], 0.0)
            nc.gpsimd.memset(ktf[2][D:P, 1, cs], 0.0)
            nc.gpsimd.memset(qtf[D:P, 1, cs], 0.0)

        ones_bc = ptile([1, D], F32, name="ones_bc")
        memset_quarter(0)
        nc.gpsimd.memset(ones_bc, 1.0)
        nc.gpsimd.memset(vones[:, :, :, D:D + 1], 1.0)
        nc.gpsimd.memset(agTb, 0.0)
        nc.gpsimd.memset(agTb[0:1, :, :], 1.0)

        # pre-warm the ACT exp table
        warm = div_sb.tile([P, 1], F32, name="warm", tag="warm")
        nc.scalar.activation(warm, bqa_t, mybir.ActivationFunctionType.Exp)

        a2a_in = [a2a_dram.tile([NCORES, D, QCW], BF16, name=f"a2a_in{h}",
                                tag=f"a2a_in{h}") for h in range(HPC)]
        a2a_out = [a2a_dram.tile([NCORES * D * QCW, 1], BF16,
                                 name=f"a2a_out{h}", tag=f"a2a_out{h}")
                   for h in range(HPC)]

        # two PSUM pools: S tiles and PV accumulators; released separately
        # so the projection can take over the S banks at the last exp
        ps = tc.alloc_tile_pool(name="ps", bufs=2, space="PSUM")

        # ---- QKV micro-units, injected into attention kc-slots ----
        def qa_unit(qc):
            cs = slice(qc * QCW, (qc + 1) * QCW)

            def emit():
                t = ps.tile([P, QGW], F32, name="tu", tag="pss")
                psq = t[:, 0:QCW]
                for cc in range(CCHUNKS):
                    nc.tensor.matmul(psq, wqa_s[:, cc, :], xbT[:, cc, cs],
                                     start=(cc == 0), stop=(cc == CCHUNKS - 1))
                nc.vector.tensor_scalar_add(qtf[:, 0, cs], psq, bqa_t)
            return emit

        def qb_unit(qc):
            cs = slice(qc * QCW, (qc + 1) * QCW)

            def emit():
                t = ps.tile([P, QGW], F32, name="tu", tag="pss")
                psq = t[0:D, 0:QCW]
                for cc in range(CCHUNKS):
                    nc.tensor.matmul(psq, wqb_s[:, cc, :], xbT[:, cc, cs],
                                     start=(cc == 0), stop=(cc == CCHUNKS - 1))
                nc.vector.tensor_scalar_add(qtf[0:D, 1, cs], psq, bqb_t)
            return emit

        def kb_unit(qc):
            # head-2's K tail: only needed once (2, *) blocks run
            cs = slice(qc * QCW, (qc + 1) * QCW)

            def emit():
                t = ps.tile([P, QGW], F32, name="tu", tag="pss")
                psk = t[0:D, 0:QCW]
                for cc in range(CCHUNKS):
                    nc.tensor.matmul(psk, wkb_s[:, cc, :], xbT[:, cc, cs],
                                     start=(cc == 0), stop=(cc == CCHUNKS - 1))
                nc.vector.tensor_scalar_add(ktf[2][0:D, 1, cs], psk, bkb_t)
            return emit

        def ka_unit(qc):
            cs = slice(qc * QCW, (qc + 1) * QCW)

            def emit():
                t = ps.tile([P, QGW], F32, name="tu", tag="pss")
                psk = t[:, 0:QCW]
                for cc in range(CCHUNKS):
                    nc.tensor.matmul(psk, wka_s[:, cc, :], xbT[:, cc, cs],
                                     start=(cc == 0),
                                     stop=(cc == CCHUNKS - 1))
                nc.vector.tensor_scalar_add(ktf[0][0:D, 0, cs],
                                            psk[0:D, :], bka_t[0:D])
                nc.vector.tensor_scalar_add(ktf[1][D:P, 0, cs],
                                            psk[D:P, :], bka_t[D:P])
            return emit

        def v_unit(rc):
            def emit():
                t = ps.tile([P, QGW], F32, name="tu", tag="pss")
                psv = t[:, 0:JC]
                for cc in range(CCHUNKS):
                    nc.tensor.matmul(psv, xbT[:, cc, rc * P:(rc + 1) * P],
                                     wva_s[:, cc, :], start=(cc == 0),
                                     stop=(cc == CCHUNKS - 1))
                nc.vector.tensor_add(
                    vones[:, rc, :, 0:D],
                    psv.rearrange("p (h d) -> p h d", h=HPC),
                    bv_bc.rearrange("p (h d) -> p h d", h=HPC))
            return emit

        class Feeder:
            """Deadline-sorted QKV unit queue; advance() emits overdue units
            plus at most ~one slot's worth of PE slack (budget in ns) of
            soon-due units, so PE bursts never starve ACT."""

            def __init__(self):
                self.units = []  # (deadline_slot, cost_ns, emit_fn)
                self.i = 0

            def seal(self):
                self.units.sort(key=lambda u: u[0])

            def advance(self, slot, budget=800.0, horizon=12):
                spent = 0.0
                while self.i < len(self.units):
                    dl, cost, emit = self.units[self.i]
                    if dl >= slot and (dl > slot + horizon or spent >= budget):
                        break
                    emit()
                    spent += cost
                    self.i += 1

            def flush(self):
                while self.i < len(self.units):
                    self.units[self.i][2]()
                    self.i += 1

        feeder = Feeder()

        pending_div = [None]

        def flush_div():
            if pending_div[0] is not None:
                pending_div[0]()
                pending_div[0] = None

        def emit_att(h, qg, slot0=None, fastdiv=False):
            pso = ps.tile([D + 1, QGW], F32, name="pso", tag="pso")
            nkc = (qg + 1) * KPG
            diag0 = qg * KPG

            def emit_pv(kc_e, qoff, pT):
                for half in range(2):
                    kc = kc_e + half
                    for sub in range(QGW // QCW):
                        lo, hi = max(qoff, sub * QCW), (sub + 1) * QCW
                        if lo >= hi:
                            continue
                        nc.tensor.matmul(
                            pso[:, lo:hi], vones[:, kc, h, :],
                            pT[:, half, lo:hi],
                            start=(kc == 0), stop=(kc == nkc - 1))

            pending = None  # PV is emitted one pair late so PE is never
            for kcp in range(nkc // 2):  # stuck waiting on the Pool masks
                kc_e = 2 * kcp
                if kcp == 2:
                    flush_div()  # prev block's atile mul: its DRAM-bounced
                    # broadcast has landed by now, so DVE does not stall
                if slot0 is not None:
                    feeder.advance(slot0 + kc_e)
                qoff = max(0, (kc_e - diag0) * P)
                pT = att_sb.tile([P, 2, QGW], BF16, name="pT", tag="pT")
                for half in range(2):
                    kc = kc_e + half
                    pss = ps.tile([P, QGW], F32, name="pss", tag="pss")
                    for sub in range(QGW // QCW):
                        lo, hi = max(qoff, sub * QCW), (sub + 1) * QCW
                        if lo >= hi:
                            continue
                        nc.tensor.matmul(
                            pss[:, lo:hi], ktf[h][:, :, kc * P:(kc + 1) * P],
                            qtf[:, :, qg * QGW + lo:qg * QGW + hi],
                            start=True, stop=True, perf_mode=DR)
                    nc.scalar.activation(
                        pT[:, half, qoff:QGW], pss[:, qoff:QGW],
                        mybir.ActivationFunctionType.Exp, scale=SCALE)
                if kc_e >= diag0:
                    # causal trim: even chunk triangle; odd chunk
                    # [zeros | triangle] over 256 cols
                    nc.gpsimd.tensor_mul(pT[:, 0, qoff:qoff + P],
                                         pT[:, 0, qoff:qoff + P], trimask)
                    nc.gpsimd.tensor_mul(pT[:, 1, qoff:qoff + 2 * P],
                                         pT[:, 1, qoff:qoff + 2 * P], m2mask)
                if pending is not None:
                    emit_pv(*pending)
                pending = (kc_e, qoff, pT)
            emit_pv(*pending)
            recip = div_sb.tile([1, QGW], F32, name="recip", tag="recip")
            nc.vector.reciprocal(recip, pso[D:D + 1, :])
            if fastdiv:
                # latency-critical division (feeds a collective): broadcast
                # across partitions with a ones-column PE matmul instead of
                # the DRAM bounce (saves ~5us on the tail)
                pbc = ps.tile([P, QGW], F32, name="pbc", tag="pss")
                for lo, hi in ((0, QCW), (QCW, QGW)):
                    nc.tensor.matmul(pbc[0:D, lo:hi], ones_bc,
                                     recip[:, lo:hi], start=True, stop=True)
                araw = div_sb.tile([D, QGW], F32, name="araw", tag="rbc")
                nc.scalar.copy(araw, pso[0:D, :])  # ACT, parallel to recip
                atile = div_sb.tile([D, QGW], BF16, name="atile", tag="atile")
                nc.vector.tensor_mul(atile, araw, pbc[0:D, :])
                for half in range(2):
                    nc.sync.dma_start(
                        a2a_in[h][2 * qg + half, :, :],
                        atile[:, half * QCW:(half + 1) * QCW])
                return
            rdram = div_dram.tile([1, QGW], F32, name="rdram", tag="rdram")
            nc.sync.dma_start(rdram, recip)
            rbc = div_sb.tile([D, QGW], F32, name="rbc", tag="rbc")
            nc.sync.dma_start(rbc, rdram.to_broadcast((D, QGW)))

            def div_mul(h=h, qg=qg, pso=pso, rbc=rbc):
                atile = div_sb.tile([D, QGW], BF16, name="atile", tag="atile")
                nc.vector.tensor_mul(atile, pso[0:D, :], rbc)
                for half in range(2):
                    nc.sync.dma_start(
                        a2a_in[h][2 * qg + half, :, :],
                        atile[:, half * QCW:(half + 1) * QCW])
            pending_div[0] = div_mul

        def emit_a2a(h):
            nc.gpsimd.collective_compute(
                "AllToAll", mybir.AluOpType.bypass,
                replica_groups=[list(range(NCORES))],
                ins=[a2a_in[h].opt()], outs=[a2a_out[h].opt()])
            for b2 in range(2):
                for sp in range(2):
                    s0 = 4 * b2 + 2 * sp
                    nc.sync.dma_start(
                        agTh[h][:, b2, sp, :],
                        a2a_out[h][s0 * D * QCW:(s0 + 2) * D * QCW, 0]
                        .rearrange("(p q) -> p q", q=QCW))

        # ---- schedule: prefix QKV for the first query group, then inject
        # the remaining QKV micro-units into attention kc-slots just before
        # their deadline, so PE fills ACT-bound attention slack ----
        # PE p-state warm-up: ~4us of throwaway matmuls that only depend on
        # the first weight load, so the real prefix is costed at full clock
        jt = ps.tile([P, QGW], F32, name="jt", tag="pss")
        for _ in range(24):
            nc.tensor.matmul(jt[:, 0:P], wqa_s[:, 0, :], wqa_s[:, 0, :],
                             start=True, stop=True)

        for qc in (0, 1):
            qa_unit(qc)()
            qb_unit(qc)()
            ka_unit(qc)()
        v_unit(0)()
        v_unit(1)()

        # block order: interleave heads so head-0's QKV deadlines spread over
        # many slots; two head-2 blocks run before (0,3) for extra slack
        order = [(0, 0), (1, 0), (0, 1), (1, 1), (0, 2), (1, 2), (2, 0),
                 (2, 1), (0, 3), "a2a0", (1, 3), "a2a1", (2, 2), (2, 3),
                 "a2a2"]
        start_slot = {}
        s = 0
        for blk in order:
            if isinstance(blk, tuple):
                start_slot[blk] = s
                s += (blk[1] + 1) * KPG
        QK_NS, V_NS = 1278.0, 480.0
        for qc in range(2, NQC):
            # stagger the Q units backward from the block-start deadline
            dl = start_slot[(0, qc // 2)] - 1
            feeder.units.append((dl - 6 * (1 - qc % 2) - 9, QK_NS,
                                 qa_unit(qc)))
            feeder.units.append((dl - 6 * (1 - qc % 2) - 6, QK_NS,
                                 qb_unit(qc)))
            feeder.units.append((start_slot[(0, qc // 2)] + 4 * qc - 7, QK_NS,
                                 ka_unit(qc)))
        for qc in range(NQC):
            feeder.units.append((start_slot[(2, qc // 2)] + 4 * qc - 7, QK_NS,
                                 kb_unit(qc)))
        for rc in range(2, RCHUNKS):
            feeder.units.append((start_slot[(0, rc // KPG)] + max(rc - 7, -1),
                                 V_NS, v_unit(rc)))
        feeder.seal()

        for blk in order:
            if blk == "a2a0":
                flush_div()
                emit_a2a(0)
            elif blk == "a2a1":
                flush_div()
                emit_a2a(1)
            elif blk == "a2a2":
                flush_div()
                emit_a2a(2)
            else:
                h, qg = blk
                if blk[0] == 0 and blk[1] >= 1:
                    memset_quarter(blk[1])
                feeder.advance(start_slot[blk] - 1)
                emit_att(h, qg, slot0=start_slot[blk],
                         fastdiv=(qg == NQG - 1))
        feeder.flush()
        ps.release()

        # wpt is only needed by the projection; load it out of the hot path
        nc.scalar.dma_start(wpt_s, wpt)

        # ---------- output projection (bias via ones-row chunk 6) ----------
        ps_pj = tc.alloc_tile_pool(name="ps_pj", bufs=2, space="PSUM")
        tiles = [(b2, rc) for b2 in range(2) for rc in range(QCW // P)]

        # short PE ramp-up before pass A (it dispatches into an idle PE)
        jt1 = ps_pj.tile([P, C], F32, name="jt1", tag="psj")
        for _ in range(20):
            nc.tensor.matmul(jt1[:, 0:P], wpt_s[:, 0, 0:P], wpt_s[:, 0, 0:P],
                             start=True, stop=True)

        # pass A: bias + heads 0/1 chunks -> bf16 partials, overlapped with
        # the last division chain and the final AllToAll
        partials = []
        for t_i, (b2, rc) in enumerate(tiles):
            psj = ps_pj.tile([P, C], F32, name="psjA", tag="psj")
            qs = slice(rc * P, (rc + 1) * P)
            for ki, k in enumerate([6, 0, 1, 2, 3]):
                st, sp = (ki == 0), (ki == 4)
                lhsT = (agTb[:, b2, qs] if k == 6
                        else agTh[k // 2][:, b2, k % 2, qs])
                for lo, hi in ((0, QCW), (QCW, C)):
                    nc.tensor.matmul(psj[:, lo:hi], lhsT,
                                     wpt_s[:, k, lo:hi], start=st, stop=sp)
            part = persist.tile([P, C], BF16, name=f"part{t_i}",
                                tag=f"part{t_i}")
            if t_i % 2:
                nc.scalar.copy(part, psj)
            else:
                nc.vector.tensor_copy(part, psj)
            partials.append(part)

        # keep PE's p-state hot through the collective flight
        jt2 = ps_pj.tile([P, C], F32, name="jt2", tag="psj")
        for _ in range(82):
            nc.tensor.matmul(jt2[:, 0:QCW], wpt_s[:, 0, 0:P],
                             wpt_s[:, 0, 0:QCW], start=True, stop=True)

        # pass B: head-2 chunks + partial + store
        for t_i, (b2, rc) in enumerate(tiles):
            psj = ps_pj.tile([P, C], F32, name="psjB", tag="psj")
            qs = slice(rc * P, (rc + 1) * P)
            for ki, k in enumerate([4, 5]):
                st, sp = (ki == 0), (ki == 1)
                lhsT = agTh[2][:, b2, k % 2, qs]
                for lo, hi in ((0, QCW), (QCW, C)):
                    nc.tensor.matmul(psj[:, lo:hi], lhsT,
                                     wpt_s[:, k, lo:hi], start=st, stop=sp)
            osb = proj_sb.tile([P, C], F32, name="osb", tag="osb")
            nc.vector.tensor_add(osb, psj, partials[t_i])
            row0 = b2 * QCW + rc * P
            eng = nc.sync if t_i % 2 else nc.scalar
            eng.dma_start(out[row0:row0 + P, :], osb)
        ps_pj.release()

    nc.compile()
    return nc


def _prep_core_inputs(x, Wq, Wk, Wv, Wp, bq, bk, bv, bp):
    """Host-side transposes/folds shared across cores, then per-core dicts."""
    xbt = []
    for b in range(B):
        xt = x[b].T.reshape(CCHUNKS, P, T).transpose(1, 0, 2)
        xbt.append(np.ascontiguousarray(xt.astype(BF16NP)))

    def fold_w(w):  # w [features, C] -> [128, CCHUNKS, features]
        wt = w.T.reshape(CCHUNKS, P, w.shape[0]).transpose(1, 0, 2)
        return np.ascontiguousarray(wt.astype(BF16NP))

    tri_np = np.triu(np.ones((P, P), dtype=np.float32)).astype(BF16NP)
    mask2_np = np.concatenate(
        [np.zeros((P, P), dtype=np.float32),
         np.triu(np.ones((P, P), dtype=np.float32))], axis=1).astype(BF16NP)

    in_maps = []
    for core in range(NCORES):
        b, hg = core // GROUPS, core % GROUPS
        js = slice(JC * hg, JC * (hg + 1))
        wq_c, wk_c, wv_c = Wq[js], Wk[js], Wv[js]
        # wpt: [p = 64*half + d, k, c]; k = 2*h_local + sp ->
        #   global head g = 3*(2*sp+half) + h_local (within the batch);
        #   chunk 6 row 0 = bp.
        wpt = np.zeros((P, 7, C), dtype=np.float32)
        for k in range(6):
            h_local, sp = k // 2, k % 2
            for half in range(2):
                g = HPC * (2 * sp + half) + h_local
                wpt[half * D:(half + 1) * D, k, :] = Wp[:, D * g:D * (g + 1)].T
        wpt[0, 6, :] = bp
        in_maps.append({
            "xbt": xbt[b],
            "wqa": fold_w(wq_c[0:P]),
            "wka": fold_w(wk_c[0:P]),
            "wqb": fold_w(wq_c[P:JC]), "wkb": fold_w(wk_c[P:JC]),
            "wva": fold_w(wv_c),
            "wpt": np.ascontiguousarray(wpt.astype(BF16NP)),
            "bqa": np.ascontiguousarray(bq[js][0:P]),
            "bqb": np.ascontiguousarray(bq[js][P:JC]),
            "bka": np.ascontiguousarray(bk[js][0:P]),
            "bkb": np.ascontiguousarray(bk[js][P:JC]),
            "bv": np.ascontiguousarray(bv[js]),
            "tri": tri_np, "mask2": mask2_np,
        })
    return in_maps


def kernel(**inputs) -> np.ndarray:
    global LAST_RESULTS
    f32 = lambda k: np.ascontiguousarray(np.asarray(inputs[k], dtype=np.float32))
    x, Wq, Wk, Wv, Wp = f32("x"), f32("Wq"), f32("Wk"), f32("Wv"), f32("Wp")
    bq, bk, bv, bp = f32("bq"), f32("bk"), f32("bv"), f32("bp")

    if "nc" not in _CACHE:
        _CACHE["nc"] = _build()
    nc = _CACHE["nc"]

    in_maps = _prep_core_inputs(x, Wq, Wk, Wv, Wp, bq, bk, bv, bp)
    res = run_bass_kernel_spmd(nc, in_maps, core_ids=list(range(NCORES)))
    LAST_RESULTS = res

    out = np.empty((B, T, C), dtype=np.float32)
    for core in range(NCORES):
        part = res.results[core]["out_part"]
        out[0, core * QCW:(core + 1) * QCW, :] = part[:QCW]
        out[1, core * QCW:(core + 1) * QCW, :] = part[QCW:]
    return out
